# revision 134
# baseline (speedup 1.0000x reference)
"""Trainium2 Bass kernel for nn_MultiHeadAttention_75548474736720.

Linear-attention-style MHA with causal prefix sums (see reference.py):
  A1 = elu(qh ph^T) + 1                     [s,j] per (b,h)
  U  = (tril(qh kh^T)/idx) @ A1 ; W = softmax_j(U)
  S2 = tril(W A1^T) ; out = (S2 @ vh)/idx reshaped @ wc + b

Sharding: 8 cores = (batch 2) x (head-group 4, 4 heads each); host sums
the row-sliced output projections per batch.

Key structure (v2, fp8 DoubleRow):
  - Projections + A1/SqT matmuls stay bf16; a1/sqT are STORED fp8(e4m3).
  - sqT carries 16*score/(t+1) (invidx folded via a pre-scaled q~hT),
    so exp(U/16) needs no per-t scale.
  - U is computed TRANSPOSED ([j,t]) with fp8 DoubleRow matmuls (2
    k-tiles per instr at 0.5 cyc/row): the exp then emits W^T directly
    -- no W transposes, no per-t scale problem.  Softmax shift
    invariance makes the +1 in A1 harmless in every route (it adds a
    j-independent per-t shift absorbed by the denominator).
  - t<128 (the only rows with large logits, ~19) go through an exact
    bf16 path: U0 = sqT0^T a1b0, per-row max subtraction (per-partition
    ACT bias), exp -> fp8 W0, PE-transposed into wtT columns.
  - den[t] = sum_j W^T via DoubleRow matmuls against a ones column
    (out free size 1 => ~free); gsc = invidx/den folds all
    normalization into the O-stage ACT scale (per-t scale invariance).
  - S2^T = a1T x wtT with fp8 DoubleRow; O/out-projection stay bf16.
"""

import sys

sys.path.insert(0, "/opt/trn_rl_repo")

import ml_dtypes
import numpy as np

import concourse.bass as bass  # noqa: F401  (registers AP machinery)
import concourse.mybir as mybir
from concourse import bacc
from concourse.tile import TileContext
from concourse.bass_utils import run_bass_kernel_spmd

F32 = mybir.dt.float32
BF16 = mybir.dt.bfloat16
F8 = mybir.dt.float8e4
ACTF = mybir.ActivationFunctionType
ALU = mybir.AluOpType
AXL = mybir.AxisListType
DR = mybir.MatmulPerfMode.DoubleRow
NPBF = ml_dtypes.bfloat16
NPF8 = ml_dtypes.float8_e4m3

B, S, DM, H = 2, 1024, 1024, 16
D = DM // H            # 64, head dim
HG = 4                 # heads per core
DL = HG * D            # 256, local dm slice
NB = S // 128          # 8 s-blocks
NORM_D = 0.125         # 1/sqrt(D)

# compact sqT layout: per s-block m, columns stored from t = 512*(m//4)
SQBASE = [0, 1024, 2048, 3072, 4096, 4608, 5120, 5632]  # total 6144
# compact s2T layout: per s-block m, columns stored from t = 128*m
S2BASE = [0, 1024, 1920, 2688, 3328, 3840, 4224, 4480]  # total 4608
S2TOT = 4608

DEBUG = False


def _sq_off(m, t0):
    return SQBASE[m] + t0 - 512 * (m // 4)


def _build_program():
    nc = bacc.Bacc(None, target_bir_lowering=False)

    qT_in = nc.declare_dram_parameter("qT", [DM, S], BF16, isOutput=False)
    kT_in = nc.declare_dram_parameter("kT", [DM, S], BF16, isOutput=False)
    vT_in = nc.declare_dram_parameter("vT", [DM, S], BF16, isOutput=False)
    pT_in = nc.declare_dram_parameter("pT", [DL, S], BF16, isOutput=False)
    wq_in = nc.declare_dram_parameter("wq", [DM, DL], BF16, isOutput=False)
    wk_in = nc.declare_dram_parameter("wk", [DM, DL], BF16, isOutput=False)
    wv_in = nc.declare_dram_parameter("wv", [DM, DL], BF16, isOutput=False)
    wc_in = nc.declare_dram_parameter("wc", [DL, S], BF16, isOutput=False)
    wqb_in = nc.declare_dram_parameter("wqb", [128, 2], F32, isOutput=False)
    wkb_in = nc.declare_dram_parameter("wkb", [128, 2], F32, isOutput=False)
    wvb_in = nc.declare_dram_parameter("wvb", [1, DL], BF16, isOutput=False)
    ones_in = nc.declare_dram_parameter("ones1", [1, 128], BF16, isOutput=False)
    ones8_in = nc.declare_dram_parameter("ones8", [128, 2], F8, isOutput=False)
    mask_in = nc.declare_dram_parameter("mask4", [4, 128, 512], BF16, isOutput=False)
    ident_in = nc.declare_dram_parameter("ident", [128, 128], BF16, isOutput=False)
    ident8_in = nc.declare_dram_parameter("ident8", [128, 128], F8, isOutput=False)
    inv_in = nc.declare_dram_parameter("invidx", [128, NB], F32, isOutput=False)
    invrow_in = nc.declare_dram_parameter("invrow", [128, 2, 512], BF16,
                                          isOutput=False)
    out_d = nc.declare_dram_parameter("out", [S, DM], BF16, isOutput=True)
    dbg = {}
    if DEBUG:
        dbg["a1"] = nc.declare_dram_parameter("d_a1", [128, NB * S], F8, isOutput=True)
        dbg["sqT"] = nc.declare_dram_parameter("d_sqT", [128, 6144], F8, isOutput=True)
        dbg["wtT"] = nc.declare_dram_parameter("d_wtT", [128, NB * S], F8, isOutput=True)
        dbg["den"] = nc.declare_dram_parameter("d_den", [128, NB], F32, isOutput=True)
        dbg["s2c"] = nc.declare_dram_parameter("d_s2c", [128, S2TOT], BF16, isOutput=True)
        dbg["w0"] = nc.declare_dram_parameter("d_w0", [128, S], F8, isOutput=True)

    with TileContext(nc) as tc:
        with tc.tile_pool(name="persist", bufs=1) as cp, \
             tc.tile_pool(name="ppm", bufs=4, space="PSUM") as ppm, \
             tc.tile_pool(name="ppt", bufs=2, space="PSUM") as ppt:

            mask = cp.tile([128, 4, 512], BF16)
            ident = cp.tile([128, 128], BF16)
            ident8 = cp.tile([128, 128], F8)
            invidx = cp.tile([128, NB], F32)
            invrow = cp.tile([128, 2, 512], BF16)
            wqb = cp.tile([128, 2], F32)
            wkb = cp.tile([128, 2], F32)
            wvb = cp.tile([1, DL], BF16)
            ones1 = cp.tile([1, 128], BF16)
            ones8 = cp.tile([128, 2, 1], F8)
            pTt = cp.tile([128, 2, S], BF16)
            qhT = cp.tile([128, 2, S], BF16)
            qsT = cp.tile([128, 2, S], BF16)   # q~ = qh * 16/(t+1)
            khT = cp.tile([128, 2, S], BF16)
            vh = cp.tile([128, NB, DL], BF16)
            oT = cp.tile([128, 2, S], BF16)
            wct = cp.tile([128, 2, S], BF16)

            # ---------------- projections ----------------
            vp_cm = tc.tile_pool(name="vproj", bufs=1)
            vp = vp_cm.__enter__()
            wvt = vp.tile([128, NB, DL], BF16)
            vTt = vp.tile([128, NB, S], BF16)
            jp_cm = tc.tile_pool(name="proj", bufs=1)
            jp = jp_cm.__enter__()
            if True:
                wqt = jp.tile([128, NB, DL], BF16)
                wkt = jp.tile([128, NB, DL], BF16)
                qTt = jp.tile([128, NB, S], BF16)
                kTt = jp.tile([128, NB, S], BF16)
                # The DMA transfer pipe is serial (exclusive DMA_ENGINES,
                # FIFO in issue order), so issue in strict priority order
                # with few, large descriptors: q first, then p (a1 needs
                # it), then k, then v/consts.
                def load_wx(wt_, wsrc, xt_, xsrc):
                    # weight whole, activations split by t-half so the first
                    # projection matmuls can start on half the data
                    nc.sync.dma_start(
                        out=wt_[:], in_=wsrc.rearrange("(a p) d -> p a d", p=128))
                    for nh in range(2):
                        nc.sync.dma_start(
                            out=xt_[:, :, nh * 512:(nh + 1) * 512],
                            in_=xsrc[:, nh * 512:(nh + 1) * 512].rearrange(
                                "(a p) t -> p a t", p=128))

                nc.sync.dma_start(
                    out=wqt[:, :, 0:128],
                    in_=wq_in[:, 0:128].rearrange("(a p) d -> p a d", p=128))
                nc.sync.dma_start(
                    out=qTt[:, :, 0:512],
                    in_=qT_in[:, 0:512].rearrange("(a p) t -> p a t", p=128))
                nc.sync.dma_start(
                    out=pTt[:], in_=pT_in.rearrange("(g p) t -> p g t", p=128))
                nc.sync.dma_start(out=wqb[:], in_=wqb_in[:])
                nc.sync.dma_start(out=invrow[:], in_=invrow_in[:])
                nc.sync.dma_start(
                    out=qTt[:, :, 512:1024],
                    in_=qT_in[:, 512:1024].rearrange("(a p) t -> p a t", p=128))
                nc.sync.dma_start(
                    out=wqt[:, :, 128:256],
                    in_=wq_in[:, 128:256].rearrange("(a p) d -> p a d", p=128))
                load_wx(wkt, wk_in, kTt, kT_in)
                nc.sync.dma_start(out=wkb[:], in_=wkb_in[:])
                nc.sync.dma_start(
                    out=mask[:], in_=mask_in.rearrange("r p c -> p r c"))
                nc.sync.dma_start(out=invidx[:], in_=inv_in[:])
                # tiny bias consts BEFORE the 2.5MB v loads: the vh matmul
                # groups end with the ones1 x wvb bias matmul, which must not
                # head-of-line block PE behind the vT transfers
                nc.sync.dma_start(out=wvb[:], in_=wvb_in[:])
                nc.sync.dma_start(out=ones1[:], in_=ones_in[:])
                load_wx(wvt, wv_in, vTt, vT_in)
                nc.sync.dma_start(out=ident8[:], in_=ident8_in[:])
                nc.sync.dma_start(
                    out=ones8[:], in_=ones8_in.rearrange("p (a o) -> p a o", o=1))
                nc.sync.dma_start(out=ident[:], in_=ident_in[:])
                nc.sync.dma_start(
                    out=wct[:], in_=wc_in.rearrange("(a p) t -> p a t", p=128))

            # ---------------- attention (4 heads) ----------------
            with tc.tile_pool(name="attn", bufs=2) as ap, \
                 tc.tile_pool(name="scr", bufs=2) as sp:
                st = {}

                def gen_a1_sq(h):
                    """a1 = elu(x)+1 (fp8), sqT = 16*score*invidx masked (fp8,
                    compact layout); bf16 copies of the s-block-0 pieces feed
                    the exact t<128 softmax path."""
                    g, p0 = h // 2, (h % 2) * 64
                    a1 = ap.tile([128, NB, S], F8, tag="a1", name="a1")
                    sqT = ap.tile([128, 6144], F8, tag="sq", name="sqT")
                    a1b0 = ap.tile([128, S], BF16, tag="a1b0", name="a1b0")
                    sqT0 = ap.tile([128, 128], BF16, tag="sq0", name="sqT0")
                    st[h] = dict(a1=a1, sqT=sqT, a1b0=a1b0, sqT0=sqT0)
                    for m in range(NB):
                        for c in range(2):
                            ps = ppm.tile([128, 512], F32, tag="a1ps", bufs=2,
                                          name="ps_a1")
                            nc.tensor.matmul(
                                ps[:], qhT[p0:p0 + 64, g, m * 128:(m + 1) * 128],
                                pTt[p0:p0 + 64, g, c * 512:(c + 1) * 512],
                                start=True, stop=True)
                            e = sp.tile([128, 512], F32, tag="e", bufs=4, name="e")
                            nc.scalar.activation(e[:], ps[:], ACTF.Exp)
                            e1 = sp.tile([128, 512], F32, tag="e1", bufs=4, name="e1")
                            nc.gpsimd.tensor_scalar_min(e1[:], e[:], 1.0)
                            if m == 0:
                                # bf16 master for the exact t<128 path; Pool
                                # (SBUF-only) derives the fp8 copy from it
                                nc.vector.scalar_tensor_tensor(
                                    a1b0[:, c * 512:(c + 1) * 512], ps[:], 0.0,
                                    e1[:], ALU.max, ALU.add)
                                nc.gpsimd.tensor_copy(
                                    a1[:, m, c * 512:(c + 1) * 512],
                                    a1b0[:, c * 512:(c + 1) * 512])
                            else:
                                nc.vector.scalar_tensor_tensor(
                                    a1[:, m, c * 512:(c + 1) * 512], ps[:], 0.0,
                                    e1[:], ALU.max, ALU.add)
                            yield
                    for m in range(NB):
                        for n in range(m // 4, 2):
                            ps = ppm.tile([128, 512], F32, tag="mm", name="ps_sq")
                            nc.tensor.matmul(
                                ps[:], khT[p0:p0 + 64, g, m * 128:(m + 1) * 128],
                                qsT[p0:p0 + 64, g, n * 512:(n + 1) * 512],
                                start=True, stop=True)
                            dst = sqT[:, _sq_off(m, n * 512):_sq_off(m, n * 512) + 512]
                            if n == m // 4:
                                nc.vector.tensor_tensor(dst, ps[:], mask[:, m % 4, :],
                                                        ALU.mult)
                                if m == 0:
                                    nc.vector.tensor_tensor(
                                        sqT0[:], ps[:, 0:128], mask[:, 0, 0:128],
                                        ALU.mult)
                            else:
                                nc.scalar.activation(dst, ps[:], ACTF.Copy)
                            yield
                    if DEBUG and h == 0:
                        nc.sync.dma_start(
                            out=dbg["a1"].rearrange("p (a b) -> p a b", a=NB), in_=a1[:])
                        nc.sync.dma_start(out=dbg["sqT"][:, :], in_=sqT[:])

                def gen_chunk0(h):
                    """Exact softmax path for t<128: bf16 U0, per-row max
                    subtraction, exp -> fp8, PE transpose into wtT cols 0:128."""
                    d = st[h]
                    wtT = ap.tile([128, NB, S], F8, tag="wt", name="wtT")
                    d["wtT"] = wtT
                    psA = ppt.tile([128, S], BF16, tag="tp",
                                   name="ps_u0a")[:].bitcast(F32)
                    nc.tensor.matmul(psA, d["sqT0"][:], d["a1b0"][:, 0:512],
                                     start=True, stop=True)
                    yield
                    psB = ppt.tile([128, S], BF16, tag="tp",
                                   name="ps_u0b")[:].bitcast(F32)
                    nc.tensor.matmul(psB, d["sqT0"][:], d["a1b0"][:, 512:1024],
                                     start=True, stop=True)
                    yield
                    mxa = sp.tile([128, 1], F32, tag="mxa", name="mxa")
                    mxb = sp.tile([128, 1], F32, tag="mxb", name="mxb")
                    nc.vector.tensor_reduce(mxa[:], psA, AXL.X, ALU.max)
                    nc.vector.tensor_reduce(mxb[:], psB, AXL.X, ALU.max)
                    bias0 = sp.tile([128, 1], F32, tag="bias0", name="bias0")
                    nc.vector.tensor_tensor(bias0[:], mxa[:], mxb[:], ALU.max)
                    # exp arg = U0/16 - mx0/16 + 5  (max logit -> e^5=148 < 240)
                    nc.vector.tensor_scalar(bias0[:], bias0[:], -1.0 / 16, 5.0,
                                            ALU.mult, ALU.add)
                    w0 = ap.tile([128, S], F8, tag="w0", name="w0")
                    nc.scalar.activation(w0[:, 0:512], psA, ACTF.Exp,
                                         bias=bias0[:, 0:1], scale=1.0 / 16)
                    nc.scalar.activation(w0[:, 512:1024], psB, ACTF.Exp,
                                         bias=bias0[:, 0:1], scale=1.0 / 16)
                    yield
                    # fp8 PE transposes write on 16-bit lanes: allocate the
                    # psum as bf16 and bitcast to an element-step-2 fp8 view
                    tps0 = ppt.tile([128, S], BF16, tag="tp", name="tps0")
                    tps08 = tps0[:].bitcast(F8).rearrange("p (a o) -> p a o", o=2)
                    for jc in range(NB):
                        nc.tensor.transpose(
                            tps08[:, jc * 128:(jc + 1) * 128, 0:1],
                            w0[:, jc * 128:(jc + 1) * 128], ident8[:])
                        if jc % 2 == 1:
                            yield
                    nc.scalar.activation(
                        wtT[:, :, 0:128],
                        tps08[:, :, 0:1].rearrange("p (a b) o -> p a b o", a=NB),
                        ACTF.Copy)
                    yield
                    if DEBUG and h == 0:
                        nc.sync.dma_start(out=dbg["w0"], in_=w0[:])

                def gen_ut(h):
                    """U^T[j,t] for t>=128 via fp8 DoubleRow; exp emits W^T."""
                    d = st[h]
                    a1, sqT, wtT = d["a1"], d["sqT"], d["wtT"]
                    sqA = sqT[:].rearrange("p (b c) -> p b c", c=1024)  # blocks 0-3
                    sqB = sqT[:].rearrange("p (b c) -> p b c", c=512)   # blocks 4-7
                    for jc in range(NB):
                        jsl = slice(jc * 128, (jc + 1) * 128)
                        p1 = ppm.tile([128, 384], F32, tag="mm", name="ps_ut1")
                        nc.tensor.matmul(p1[:], a1[:, 0:2, jsl], sqA[:, 0:2, 128:512],
                                         start=True, stop=False, perf_mode=DR)
                        nc.tensor.matmul(p1[:], a1[:, 2:4, jsl], sqA[:, 2:4, 128:512],
                                         start=False, stop=True, perf_mode=DR)
                        yield
                        nc.scalar.activation(wtT[:, jc, 128:512], p1[:], ACTF.Exp,
                                             scale=1.0 / 16)
                        p2 = ppm.tile([128, 512], F32, tag="mm", name="ps_ut2")
                        nc.tensor.matmul(p2[:], a1[:, 0:2, jsl], sqA[:, 0:2, 512:1024],
                                         start=True, stop=False, perf_mode=DR)
                        nc.tensor.matmul(p2[:], a1[:, 2:4, jsl], sqA[:, 2:4, 512:1024],
                                         start=False, stop=False, perf_mode=DR)
                        nc.tensor.matmul(p2[:], a1[:, 4:6, jsl], sqB[:, 8:10, :],
                                         start=False, stop=False, perf_mode=DR)
                        nc.tensor.matmul(p2[:], a1[:, 6:8, jsl], sqB[:, 10:12, :],
                                         start=False, stop=True, perf_mode=DR)
                        yield
                        nc.scalar.activation(wtT[:, jc, 512:1024], p2[:], ACTF.Exp,
                                             scale=1.0 / 16)
                    if DEBUG and h == 0:
                        nc.sync.dma_start(
                            out=dbg["wtT"].rearrange("p (a b) -> p a b", a=NB),
                            in_=wtT[:])

                def gen_a1t(h):
                    """A1^T via PE transposes of fp8 a1 + one copy per block."""
                    d = st[h]
                    a1 = d["a1"]
                    a1T = ap.tile([128, NB, S], F8, tag="a1t", bufs=1, name="a1T")
                    d["a1T"] = a1T
                    for m in range(NB):
                        tps = ppt.tile([128, S], BF16, tag="tp", name="tps")
                        tps8 = tps[:].bitcast(F8).rearrange("p (a o) -> p a o", o=2)
                        for k in range(NB):
                            nc.tensor.transpose(
                                tps8[:, k * 128:(k + 1) * 128, 0:1],
                                a1[:, m, k * 128:(k + 1) * 128], ident8[:])
                        yield
                        src = tps8[:, :, 0:1].rearrange("p (a b) o -> p a b o", a=NB)
                        if m == 7:
                            nc.scalar.activation(
                                a1T[:, :, m * 128:(m + 1) * 128], src, ACTF.Copy)
                        else:
                            nc.vector.tensor_copy(
                                a1T[:, :, m * 128:(m + 1) * 128], src)
                        yield

                def gen_den(h):
                    """den[t] = sum_j wtT[j,t] via DoubleRow x ones (free)."""
                    d = st[h]
                    wtT = d["wtT"]
                    dps = ppm.tile([128, NB], F32, tag="mm", name="ps_den")
                    for i in range(NB):
                        for k in range(4):
                            nc.tensor.matmul(
                                dps[:, i:i + 1],
                                wtT[:, 2 * k:2 * k + 2, i * 128:(i + 1) * 128],
                                ones8[:], start=(k == 0), stop=(k == 3),
                                perf_mode=DR)
                        if i % 2 == 1:
                            yield
                    denB = sp.tile([128, NB], F32, tag="denB", name="denB")
                    nc.vector.tensor_copy(denB[:], dps[:])
                    recden = sp.tile([128, NB], F32, tag="recden", name="recden")
                    nc.vector.reciprocal(recden[:], denB[:])
                    gsc = sp.tile([128, NB], F32, tag="gsc", name="gsc")
                    nc.vector.tensor_tensor(gsc[:], recden[:], invidx[:], ALU.mult)
                    d["gsc"] = gsc
                    yield
                    if DEBUG and h == 0:
                        nc.sync.dma_start(out=dbg["den"], in_=denB[:])

                def gen_s2(h):
                    """S2^T[s,t] = sum_j A1^T[j,s] W^T[j,t] (fp8 DoubleRow),
                    tril-masked on the diagonal, stored compact bf16."""
                    d = st[h]
                    a1T, wtT = d["a1T"], d["wtT"]
                    s2c = ap.tile([128, S2TOT], BF16, tag="s2", name="s2c")
                    d["s2c"] = s2c
                    for m in range(NB):
                        msl = slice(m * 128, (m + 1) * 128)
                        if m < 4:
                            chunks = [(m * 128, 512 - m * 128), (512, 512)]
                        else:
                            chunks = [(m * 128, 1024 - m * 128)]
                        for t0, w in chunks:
                            ps = ppm.tile([128, w], F32, tag="mm", name="ps_s2")
                            for k in range(4):
                                nc.tensor.matmul(
                                    ps[:], a1T[:, 2 * k:2 * k + 2, msl],
                                    wtT[:, 2 * k:2 * k + 2, t0:t0 + w],
                                    start=(k == 0), stop=(k == 3), perf_mode=DR)
                            yield
                            base = S2BASE[m] + (t0 - m * 128)
                            if t0 == m * 128:
                                nc.vector.tensor_tensor(
                                    s2c[:, base:base + 128], ps[:, 0:128],
                                    mask[:, 0, 0:128], ALU.mult)
                                if w > 128:
                                    nc.scalar.activation(
                                        s2c[:, base + 128:base + w], ps[:, 128:w],
                                        ACTF.Copy)
                            else:
                                nc.scalar.activation(s2c[:, base:base + w], ps[:],
                                                     ACTF.Copy)
                            yield
                    if DEBUG and h == 0:
                        nc.sync.dma_start(out=dbg["s2c"], in_=s2c[:])

                oNs = {}

                def emit_o(h):
                    """O[t,d] = gsc[t] * sum_{s<=t} S2T[s,t] vh[s,d]; heads
                    pair into one oN tile; PE transpose -> oT [d,t].  For the
                    last head the transpose + output projection are pipelined
                    per t-block to shrink the tail."""
                    d = st.pop(h)
                    s2c, gsc = d["s2c"], d["gsc"]
                    if h % 2 == 0:
                        oNs[h // 2] = sp.tile([128, NB, 128], BF16, tag="oN",
                                              bufs=4, name="oN")
                    oN = oNs[h // 2]
                    d0 = (h % 2) * 64
                    last = (h == HG - 1)
                    if not last:
                        # all 8 t-blocks share one psum bank (single
                        # accumulation group via the lazy zero-region), then
                        # one DVE multiply against a Pool-built broadcast gsc
                        gse = sp.tile([128, NB, 64], F32, tag="gse", name="gse")
                        nc.gpsimd.tensor_copy(
                            gse[:], gsc[:].broadcast_to((128, NB, 64)))
                        ps = ppm.tile([128, 512], F32, tag="mm", name="ps_o")
                        for i in range(NB):
                            for m in range(i + 1):
                                nc.tensor.matmul(
                                    ps[:, i * 64:(i + 1) * 64],
                                    s2c[:, S2BASE[m] + (i - m) * 128:
                                        S2BASE[m] + (i - m) * 128 + 128],
                                    vh[:, m, h * 64:(h + 1) * 64],
                                    start=(i == 0 and m == 0),
                                    stop=(i == NB - 1 and m == i),
                                    skip_group_check=True)
                            if i % 2 == 1:
                                yield
                        nc.vector.tensor_tensor(
                            oN[:, :, d0:d0 + 64],
                            ps[:].rearrange("p (a d) -> p a d", a=NB),
                            gse[:], ALU.mult)
                        yield
                    for i in range(NB if last else 0):
                        ps = ppm.tile([128, 64], F32, tag="mm", name="ps_o")
                        for m in range(i + 1):
                            nc.tensor.matmul(
                                ps[:], s2c[:, S2BASE[m] + (i - m) * 128:
                                           S2BASE[m] + (i - m) * 128 + 128],
                                vh[:, m, h * 64:(h + 1) * 64],
                                start=(m == 0), stop=(m == i))
                        nc.vector.tensor_scalar(oN[:, i, d0:d0 + 64], ps[:],
                                                gsc[:, i:i + 1], None, ALU.mult)
                        if last:
                            tps = ppm.tile([128, 128], BF16, tag="a1ps", bufs=2,
                                           name="tpo")
                            nc.tensor.transpose(tps[:], oN[:, i, :], ident[:])
                            nc.scalar.activation(
                                oT[:, h // 2, i * 128:(i + 1) * 128], tps[:],
                                ACTF.Copy)
                            emit_final_tile(i)
                        yield
                    if h % 2 == 1 and not last:
                        oN = oNs.pop(h // 2)
                        tps = ppm.tile([128, S], BF16, tag="mm", name="tpo")
                        for i in range(NB):
                            nc.tensor.transpose(
                                tps[:, i * 128:(i + 1) * 128], oN[:, i, :], ident[:])
                        nc.scalar.activation(
                            oT[:, h // 2, :],
                            tps[:].rearrange("p (a b) -> p a b", a=NB), ACTF.Copy)

                def emit_final_tile(i):
                    for c in range(2):
                        ps = ppm.tile([128, 512], F32, tag="mm", name="ps_fin")
                        for g2 in range(2):
                            nc.tensor.matmul(
                                ps[:], oT[:, g2, i * 128:(i + 1) * 128],
                                wct[:, g2, c * 512:(c + 1) * 512],
                                start=(g2 == 0), stop=(g2 == 1))
                        ot = sp.tile([128, 512], BF16, tag="ot", bufs=3, name="ot")
                        nc.vector.tensor_copy(ot[:], ps[:])
                        nc.sync.dma_start(
                            out=out_d[i * 128:(i + 1) * 128, c * 512:(c + 1) * 512],
                            in_=ot[:])

                def chain(*gens):
                    for gg in gens:
                        yield from gg

                # q/k projections interleaved with head 0's A1 matmuls
                # (which need only qhT+pTt) so the PE stream is never
                # head-of-line blocked on a projection DMA
                gen0 = gen_a1_sq(0)

                def pull0(k):
                    for _ in range(k):
                        if next(gen0, "done") == "done":
                            break

                for gi, (g, n) in enumerate(((0, 0), (0, 1), (1, 0), (1, 1))):
                    ps = ppm.tile([128, 512], F32, tag="mm", name="ps_proj")
                    for kb in range(NB):
                        nc.tensor.matmul(
                            ps[:], wqt[:, kb, g * 128:(g + 1) * 128],
                            qTt[:, kb, n * 512:(n + 1) * 512],
                            start=(kb == 0), stop=(kb == NB - 1))
                    nc.scalar.activation(
                        qhT[:, g, n * 512:(n + 1) * 512], ps[:],
                        ACTF.Identity, bias=wqb[:, g:g + 1], scale=NORM_D)
                    nc.gpsimd.tensor_tensor(
                        qsT[:, g, n * 512:(n + 1) * 512],
                        qhT[:, g, n * 512:(n + 1) * 512],
                        invrow[:, n, :], ALU.mult)
                    pull0((4, 4, 2, 2)[gi])
                for g in range(2):
                    for n in range(2):
                        ps = ppm.tile([128, 512], F32, tag="mm", name="ps_projk")
                        for kb in range(NB):
                            nc.tensor.matmul(
                                ps[:], wkt[:, kb, g * 128:(g + 1) * 128],
                                kTt[:, kb, n * 512:(n + 1) * 512],
                                start=(kb == 0), stop=(kb == NB - 1))
                        nc.scalar.activation(
                            khT[:, g, n * 512:(n + 1) * 512], ps[:],
                            ACTF.Identity, bias=wkb[:, g:g + 1], scale=1.0)
                        pull0(2)

                # vh[s, d] = sum_c vT[c, s] wv[c, d] + wv_b[d], interleaved
                # with head 0's A1/SqT so PE has work while vT streams in
                for m2 in range(0, NB, 2):
                    # two s-blocks share one psum bank: the first start marks
                    # the whole 2KB zero-region, the second block accumulates
                    # into its (lazily zeroed) half -> one copy per pair
                    ps = ppm.tile([128, 2 * DL], F32, tag="mm", name="ps_vh")
                    for mo in range(2):
                        m = m2 + mo
                        psl = ps[:, mo * DL:(mo + 1) * DL]
                        for kb in range(NB):
                            nc.tensor.matmul(
                                psl, vTt[:, kb, m * 128:(m + 1) * 128],
                                wvt[:, kb, :],
                                start=(m2 == m and kb == 0 and mo == 0),
                                stop=False, skip_group_check=True)
                        nc.tensor.matmul(psl, ones1[:], wvb[:], start=False,
                                         stop=(mo == 1), skip_group_check=True)
                        pull0(3)
                    nc.scalar.activation(vh[:, m2:m2 + 2, :],
                                         ps[:].rearrange("p (a d) -> p a d", a=2),
                                         ACTF.Copy)
                for _ in gen0:
                    pass

                for h in range(HG):
                    gnext = gen_a1_sq(h + 1) if h + 1 < HG else None
                    # O(h-1) first: its inputs are long-ready, so it fills
                    # the phase-boundary bubble while head h's a1/sqT fp8
                    # chains drain
                    parts = [emit_o(h - 1)] if h >= 1 else []
                    last = h == HG - 1
                    wgen = chain(*parts, gen_chunk0(h), gen_ut(h),
                                 *([] if last else [gen_a1t(h)]), gen_den(h))
                    for wi, _ in enumerate(wgen):
                        if gnext is not None:
                            next(gnext, None)
                            if wi < 12:
                                next(gnext, None)
                    if last:
                        # tail: S2's block m only needs a1T's m-chunk, and
                        # O's t-block i only needs s2c blocks m<=i -- so a1T
                        # production, S2, O and the output projection all
                        # pipeline per-block
                        a1tg = gen_a1t(h)
                        next(a1tg, None)
                        next(a1tg, None)
                        og = emit_o(h)
                        osteps = 0
                        yields_at_block_done = [4, 8, 12, 16, 18, 20, 22, 24]
                        nyield = 0
                        for _ in gen_s2(h):
                            nyield += 1
                            next(a1tg, None)
                            blocks_done = sum(
                                1 for yy in yields_at_block_done if nyield >= yy)
                            while og is not None and osteps < blocks_done:
                                if next(og, "done") == "done":
                                    og = None
                                    break
                                osteps += 1
                        if og is not None:
                            for _ in og:
                                pass
                    else:
                        for _ in gen_s2(h):
                            if gnext is not None:
                                next(gnext, None)
                        if gnext is not None:
                            for _ in gnext:
                                pass

            jp_cm.__exit__(None, None, None)
            vp_cm.__exit__(None, None, None)

    nc.finalize()
    return nc


_CACHE = {}


def _get_program():
    if "nc" not in _CACHE:
        _CACHE["nc"] = _build_program()
    return _CACHE["nc"]


def _consts():
    if "consts" not in _CACHE:
        p_ = np.arange(128, dtype=np.float32)[:, None]
        c_ = np.arange(512, dtype=np.float32)[None, :]
        mask4 = np.stack(
            [(p_ + 128.0 * r <= c_) for r in range(4)]).astype(NPBF)
        ident = np.eye(128, dtype=np.float32).astype(NPBF)
        ident8 = np.eye(128, dtype=np.float32).astype(NPF8)
        blk = np.arange(NB, dtype=np.float32)[None, :]
        invidx = (1.0 / (blk * 128.0 + p_ + 1.0)).astype(np.float32)
        ones1 = np.ones((1, 128), NPBF)
        ones8 = np.ones((128, 2), NPF8)
        n_ = np.arange(2, dtype=np.float32)[:, None]
        c2_ = np.arange(512, dtype=np.float32)[None, :]
        invrow = np.broadcast_to(
            (16.0 / (n_ * 512.0 + c2_ + 1.0))[None, :, :],
            (128, 2, 512)).astype(NPBF)
        invrow = np.ascontiguousarray(invrow)
        _CACHE["consts"] = (mask4, ident, ident8, invidx, ones1, ones8, invrow)
    return _CACHE["consts"]


PROFILE = False
LAST_RESULTS = None


def kernel(v, k, q, p, wq_k, wq_b, wk_k, wk_b, wv_k, wv_b, wc_k, wc_b):
    global LAST_RESULTS
    nc = _get_program()
    mask4, ident, ident8, invidx, ones1, ones8, invrow = _consts()

    qT = [np.ascontiguousarray(q[b].T).astype(NPBF) for b in range(B)]
    kT = [np.ascontiguousarray(k[b].T).astype(NPBF) for b in range(B)]
    vT = [np.ascontiguousarray(v[b].T).astype(NPBF) for b in range(B)]
    pT = [np.ascontiguousarray(p[b].T).astype(NPBF) for b in range(B)]
    wqc = wq_k.astype(NPBF)
    wkc = wk_k.astype(NPBF)
    wvc = wv_k.astype(NPBF)
    wcc = wc_k.astype(NPBF)

    in_maps = []
    for c in range(8):
        b, hg = c // 4, c % 4
        c0 = hg * DL
        wqb = np.ascontiguousarray(
            (wq_b[c0:c0 + DL].reshape(2, 128).T * NORM_D).astype(np.float32))
        wkb = np.ascontiguousarray(wk_b[c0:c0 + DL].reshape(2, 128).T.astype(np.float32))
        in_maps.append({
            "qT": qT[b], "kT": kT[b], "vT": vT[b],
            "pT": np.ascontiguousarray(pT[b][c0:c0 + DL]),
            "wq": np.ascontiguousarray(wqc[:, c0:c0 + DL]),
            "wk": np.ascontiguousarray(wkc[:, c0:c0 + DL]),
            "wv": np.ascontiguousarray(wvc[:, c0:c0 + DL]),
            "wc": np.ascontiguousarray(wcc[c0:c0 + DL, :]),
            "wqb": wqb, "wkb": wkb,
            "wvb": np.ascontiguousarray(wv_b[c0:c0 + DL].reshape(1, DL).astype(NPBF)),
            "ones1": ones1, "ones8": ones8, "mask4": mask4, "ident": ident,
            "ident8": ident8, "invidx": invidx, "invrow": invrow,
        })

    res = run_bass_kernel_spmd(
        nc, in_maps, core_ids=list(range(8)), trace=PROFILE)
    LAST_RESULTS = res

    out = np.zeros((B, S, DM), np.float32)
    for c in range(8):
        out[c // 4] += res.results[c]["out"].astype(np.float32)
    out += wc_b[None, None, :].astype(np.float32)
    return out


# revision 137
# speedup vs baseline: 1.0052x; 1.0052x over previous
"""Trainium2 Bass kernel for nn_MultiHeadAttention_75548474736720.

Linear-attention-style MHA with causal prefix sums (see reference.py):
  A1 = elu(qh ph^T) + 1                     [s,j] per (b,h)
  U  = (tril(qh kh^T)/idx) @ A1 ; W = softmax_j(U)
  S2 = tril(W A1^T) ; out = (S2 @ vh)/idx reshaped @ wc + b

Sharding: 8 cores = (batch 2) x (head-group 4, 4 heads each); host sums
the row-sliced output projections per batch.

Key structure (v2, fp8 DoubleRow):
  - Projections + A1/SqT matmuls stay bf16; a1/sqT are STORED fp8(e4m3).
  - sqT carries 16*score/(t+1) (invidx folded via a pre-scaled q~hT),
    so exp(U/16) needs no per-t scale.
  - U is computed TRANSPOSED ([j,t]) with fp8 DoubleRow matmuls (2
    k-tiles per instr at 0.5 cyc/row): the exp then emits W^T directly
    -- no W transposes, no per-t scale problem.  Softmax shift
    invariance makes the +1 in A1 harmless in every route (it adds a
    j-independent per-t shift absorbed by the denominator).
  - t<128 (the only rows with large logits, ~19) go through an exact
    bf16 path: U0 = sqT0^T a1b0, per-row max subtraction (per-partition
    ACT bias), exp -> fp8 W0, PE-transposed into wtT columns.
  - den[t] = sum_j W^T via DoubleRow matmuls against a ones column
    (out free size 1 => ~free); gsc = invidx/den folds all
    normalization into the O-stage ACT scale (per-t scale invariance).
  - S2^T = a1T x wtT with fp8 DoubleRow; O/out-projection stay bf16.
"""

import sys

sys.path.insert(0, "/opt/trn_rl_repo")

import ml_dtypes
import numpy as np

import concourse.bass as bass  # noqa: F401  (registers AP machinery)
import concourse.mybir as mybir
from concourse import bacc
from concourse.tile import TileContext
from concourse.bass_utils import run_bass_kernel_spmd

F32 = mybir.dt.float32
BF16 = mybir.dt.bfloat16
F8 = mybir.dt.float8e4
ACTF = mybir.ActivationFunctionType
ALU = mybir.AluOpType
AXL = mybir.AxisListType
DR = mybir.MatmulPerfMode.DoubleRow
NPBF = ml_dtypes.bfloat16
NPF8 = ml_dtypes.float8_e4m3

B, S, DM, H = 2, 1024, 1024, 16
D = DM // H            # 64, head dim
HG = 4                 # heads per core
DL = HG * D            # 256, local dm slice
NB = S // 128          # 8 s-blocks
NORM_D = 0.125         # 1/sqrt(D)

# compact sqT layout: per s-block m, columns stored from t = 512*(m//4)
SQBASE = [0, 1024, 2048, 3072, 4096, 4608, 5120, 5632]  # total 6144
# compact s2T layout: per s-block m, columns stored from t = 128*m
S2BASE = [0, 1024, 1920, 2688, 3328, 3840, 4224, 4480]  # total 4608
S2TOT = 4608

DEBUG = False


def _sq_off(m, t0):
    return SQBASE[m] + t0 - 512 * (m // 4)


def _build_program():
    nc = bacc.Bacc(None, target_bir_lowering=False)

    qT_in = nc.declare_dram_parameter("qT", [DM, S], BF16, isOutput=False)
    kT_in = nc.declare_dram_parameter("kT", [DM, S], BF16, isOutput=False)
    vT_in = nc.declare_dram_parameter("vT", [DM, S], BF16, isOutput=False)
    pT_in = nc.declare_dram_parameter("pT", [DL, S], BF16, isOutput=False)
    wq_in = nc.declare_dram_parameter("wq", [DM, DL], BF16, isOutput=False)
    wk_in = nc.declare_dram_parameter("wk", [DM, DL], BF16, isOutput=False)
    wv_in = nc.declare_dram_parameter("wv", [DM, DL], BF16, isOutput=False)
    wc_in = nc.declare_dram_parameter("wc", [DL, S], BF16, isOutput=False)
    wqb_in = nc.declare_dram_parameter("wqb", [128, 2], F32, isOutput=False)
    wkb_in = nc.declare_dram_parameter("wkb", [128, 2], F32, isOutput=False)
    wvb_in = nc.declare_dram_parameter("wvb", [1, DL], BF16, isOutput=False)
    ones_in = nc.declare_dram_parameter("ones1", [1, 128], BF16, isOutput=False)
    ones8_in = nc.declare_dram_parameter("ones8", [128, 2], F8, isOutput=False)
    mask_in = nc.declare_dram_parameter("mask4", [4, 128, 512], BF16, isOutput=False)
    ident_in = nc.declare_dram_parameter("ident", [128, 128], BF16, isOutput=False)
    ident8_in = nc.declare_dram_parameter("ident8", [128, 128], F8, isOutput=False)
    inv_in = nc.declare_dram_parameter("invidx", [128, NB], F32, isOutput=False)
    invrow_in = nc.declare_dram_parameter("invrow", [128, 2, 512], BF16,
                                          isOutput=False)
    out_d = nc.declare_dram_parameter("out", [S, DM], BF16, isOutput=True)
    dbg = {}
    if DEBUG:
        dbg["a1"] = nc.declare_dram_parameter("d_a1", [128, NB * S], F8, isOutput=True)
        dbg["sqT"] = nc.declare_dram_parameter("d_sqT", [128, 6144], F8, isOutput=True)
        dbg["wtT"] = nc.declare_dram_parameter("d_wtT", [128, NB * S], F8, isOutput=True)
        dbg["den"] = nc.declare_dram_parameter("d_den", [128, NB], F32, isOutput=True)
        dbg["s2c"] = nc.declare_dram_parameter("d_s2c", [128, S2TOT], BF16, isOutput=True)
        dbg["w0"] = nc.declare_dram_parameter("d_w0", [128, S], F8, isOutput=True)

    with TileContext(nc) as tc:
        with tc.tile_pool(name="persist", bufs=1) as cp, \
             tc.tile_pool(name="ppm", bufs=4, space="PSUM") as ppm, \
             tc.tile_pool(name="ppt", bufs=2, space="PSUM") as ppt:

            mask = cp.tile([128, 4, 512], BF16)
            ident = cp.tile([128, 128], BF16)
            ident8 = cp.tile([128, 128], F8)
            invidx = cp.tile([128, NB], F32)
            invrow = cp.tile([128, 2, 512], BF16)
            wqb = cp.tile([128, 2], F32)
            wkb = cp.tile([128, 2], F32)
            wvb = cp.tile([1, DL], BF16)
            ones1 = cp.tile([1, 128], BF16)
            ones8 = cp.tile([128, 2, 1], F8)
            pTt = cp.tile([128, 2, S], BF16)
            qhT = cp.tile([128, 2, S], BF16)
            qsT = cp.tile([128, 2, S], BF16)   # q~ = qh * 16/(t+1)
            khT = cp.tile([128, 2, S], BF16)
            vh = cp.tile([128, NB, DL], BF16)
            oT = cp.tile([128, 2, S], BF16)
            wct = cp.tile([128, 2, S], BF16)

            # ---------------- projections ----------------
            vp_cm = tc.tile_pool(name="vproj", bufs=1)
            vp = vp_cm.__enter__()
            wvt = vp.tile([128, NB, DL], BF16)
            vTt = vp.tile([128, NB, S], BF16)
            jp_cm = tc.tile_pool(name="proj", bufs=1)
            jp = jp_cm.__enter__()
            if True:
                wqt = jp.tile([128, NB, DL], BF16)
                wkt = jp.tile([128, NB, DL], BF16)
                qTt = jp.tile([128, NB, S], BF16)
                kTt = jp.tile([128, NB, S], BF16)
                # The DMA transfer pipe is serial (exclusive DMA_ENGINES,
                # FIFO in issue order), so issue in strict priority order
                # with few, large descriptors: q first, then p (a1 needs
                # it), then k, then v/consts.
                def load_wx(wt_, wsrc, xt_, xsrc):
                    # weight whole, activations split by t-half so the first
                    # projection matmuls can start on half the data
                    nc.sync.dma_start(
                        out=wt_[:], in_=wsrc.rearrange("(a p) d -> p a d", p=128))
                    for nh in range(2):
                        nc.sync.dma_start(
                            out=xt_[:, :, nh * 512:(nh + 1) * 512],
                            in_=xsrc[:, nh * 512:(nh + 1) * 512].rearrange(
                                "(a p) t -> p a t", p=128))

                nc.sync.dma_start(
                    out=wqt[:, :, 0:128],
                    in_=wq_in[:, 0:128].rearrange("(a p) d -> p a d", p=128))
                nc.sync.dma_start(
                    out=qTt[:, :, 0:512],
                    in_=qT_in[:, 0:512].rearrange("(a p) t -> p a t", p=128))
                nc.sync.dma_start(
                    out=pTt[:], in_=pT_in.rearrange("(g p) t -> p g t", p=128))
                nc.sync.dma_start(out=wqb[:], in_=wqb_in[:])
                nc.sync.dma_start(out=invrow[:], in_=invrow_in[:])
                nc.sync.dma_start(
                    out=qTt[:, :, 512:1024],
                    in_=qT_in[:, 512:1024].rearrange("(a p) t -> p a t", p=128))
                nc.sync.dma_start(
                    out=wqt[:, :, 128:256],
                    in_=wq_in[:, 128:256].rearrange("(a p) d -> p a d", p=128))
                load_wx(wkt, wk_in, kTt, kT_in)
                nc.sync.dma_start(out=wkb[:], in_=wkb_in[:])
                nc.sync.dma_start(
                    out=mask[:], in_=mask_in.rearrange("r p c -> p r c"))
                nc.sync.dma_start(out=invidx[:], in_=inv_in[:])
                # tiny bias consts BEFORE the 2.5MB v loads: the vh matmul
                # groups end with the ones1 x wvb bias matmul, which must not
                # head-of-line block PE behind the vT transfers
                nc.sync.dma_start(out=wvb[:], in_=wvb_in[:])
                nc.sync.dma_start(out=ones1[:], in_=ones_in[:])
                load_wx(wvt, wv_in, vTt, vT_in)
                nc.sync.dma_start(out=ident8[:], in_=ident8_in[:])
                nc.sync.dma_start(
                    out=ones8[:], in_=ones8_in.rearrange("p (a o) -> p a o", o=1))
                nc.sync.dma_start(out=ident[:], in_=ident_in[:])
                nc.sync.dma_start(
                    out=wct[:], in_=wc_in.rearrange("(a p) t -> p a t", p=128))

            # ---------------- attention (4 heads) ----------------
            with tc.tile_pool(name="attn", bufs=2) as ap, \
                 tc.tile_pool(name="scr", bufs=2) as sp:
                st = {}

                def gen_a1_sq(h):
                    """a1 = elu(x)+1 (fp8), sqT = 16*score*invidx masked (fp8,
                    compact layout); bf16 copies of the s-block-0 pieces feed
                    the exact t<128 softmax path."""
                    g, p0 = h // 2, (h % 2) * 64
                    a1 = ap.tile([128, NB, S], F8, tag="a1", name="a1")
                    sqT = ap.tile([128, 6144], F8, tag="sq", name="sqT")
                    a1b0 = ap.tile([128, S], BF16, tag="a1b0", name="a1b0")
                    sqT0 = ap.tile([128, 128], BF16, tag="sq0", name="sqT0")
                    st[h] = dict(a1=a1, sqT=sqT, a1b0=a1b0, sqT0=sqT0)
                    for m in range(NB):
                        for c in range(2):
                            ps = ppm.tile([128, 512], F32, tag="a1ps", bufs=2,
                                          name="ps_a1")
                            nc.tensor.matmul(
                                ps[:], qhT[p0:p0 + 64, g, m * 128:(m + 1) * 128],
                                pTt[p0:p0 + 64, g, c * 512:(c + 1) * 512],
                                start=True, stop=True)
                            e = sp.tile([128, 512], F32, tag="e", bufs=4, name="e")
                            nc.scalar.activation(e[:], ps[:], ACTF.Exp)
                            e1 = sp.tile([128, 512], F32, tag="e1", bufs=4, name="e1")
                            nc.gpsimd.tensor_scalar_min(e1[:], e[:], 1.0)
                            if m == 0:
                                # bf16 master for the exact t<128 path; Pool
                                # (SBUF-only) derives the fp8 copy from it
                                nc.vector.scalar_tensor_tensor(
                                    a1b0[:, c * 512:(c + 1) * 512], ps[:], 0.0,
                                    e1[:], ALU.max, ALU.add)
                                nc.gpsimd.tensor_copy(
                                    a1[:, m, c * 512:(c + 1) * 512],
                                    a1b0[:, c * 512:(c + 1) * 512])
                            else:
                                nc.vector.scalar_tensor_tensor(
                                    a1[:, m, c * 512:(c + 1) * 512], ps[:], 0.0,
                                    e1[:], ALU.max, ALU.add)
                            yield
                    for m in range(NB):
                        for n in range(m // 4, 2):
                            ps = ppm.tile([128, 512], F32, tag="mm", name="ps_sq")
                            nc.tensor.matmul(
                                ps[:], khT[p0:p0 + 64, g, m * 128:(m + 1) * 128],
                                qsT[p0:p0 + 64, g, n * 512:(n + 1) * 512],
                                start=True, stop=True)
                            dst = sqT[:, _sq_off(m, n * 512):_sq_off(m, n * 512) + 512]
                            if n == m // 4:
                                nc.vector.tensor_tensor(dst, ps[:], mask[:, m % 4, :],
                                                        ALU.mult)
                                if m == 0:
                                    nc.vector.tensor_tensor(
                                        sqT0[:], ps[:, 0:128], mask[:, 0, 0:128],
                                        ALU.mult)
                            else:
                                nc.scalar.activation(dst, ps[:], ACTF.Copy)
                            yield
                    if DEBUG and h == 0:
                        nc.sync.dma_start(
                            out=dbg["a1"].rearrange("p (a b) -> p a b", a=NB), in_=a1[:])
                        nc.sync.dma_start(out=dbg["sqT"][:, :], in_=sqT[:])

                def gen_chunk0(h):
                    """Exact softmax path for t<128: bf16 U0, per-row max
                    subtraction, exp -> fp8, PE transpose into wtT cols 0:128."""
                    d = st[h]
                    wtT = ap.tile([128, NB, S], F8, tag="wt", name="wtT")
                    d["wtT"] = wtT
                    psA = ppt.tile([128, S], BF16, tag="tp",
                                   name="ps_u0a")[:].bitcast(F32)
                    nc.tensor.matmul(psA, d["sqT0"][:], d["a1b0"][:, 0:512],
                                     start=True, stop=True)
                    yield
                    psB = ppt.tile([128, S], BF16, tag="tp",
                                   name="ps_u0b")[:].bitcast(F32)
                    nc.tensor.matmul(psB, d["sqT0"][:], d["a1b0"][:, 512:1024],
                                     start=True, stop=True)
                    yield
                    mxa = sp.tile([128, 1], F32, tag="mxa", name="mxa")
                    mxb = sp.tile([128, 1], F32, tag="mxb", name="mxb")
                    nc.vector.tensor_reduce(mxa[:], psA, AXL.X, ALU.max)
                    nc.vector.tensor_reduce(mxb[:], psB, AXL.X, ALU.max)
                    bias0 = sp.tile([128, 1], F32, tag="bias0", name="bias0")
                    nc.vector.tensor_tensor(bias0[:], mxa[:], mxb[:], ALU.max)
                    # exp arg = U0/16 - mx0/16 + 5  (max logit -> e^5=148 < 240)
                    nc.vector.tensor_scalar(bias0[:], bias0[:], -1.0 / 16, 5.0,
                                            ALU.mult, ALU.add)
                    w0 = ap.tile([128, S], F8, tag="w0", name="w0")
                    nc.scalar.activation(w0[:, 0:512], psA, ACTF.Exp,
                                         bias=bias0[:, 0:1], scale=1.0 / 16)
                    nc.scalar.activation(w0[:, 512:1024], psB, ACTF.Exp,
                                         bias=bias0[:, 0:1], scale=1.0 / 16)
                    yield
                    # fp8 PE transposes write on 16-bit lanes: allocate the
                    # psum as bf16 and bitcast to an element-step-2 fp8 view
                    tps0 = ppt.tile([128, S], BF16, tag="tp", name="tps0")
                    tps08 = tps0[:].bitcast(F8).rearrange("p (a o) -> p a o", o=2)
                    for jc in range(NB):
                        nc.tensor.transpose(
                            tps08[:, jc * 128:(jc + 1) * 128, 0:1],
                            w0[:, jc * 128:(jc + 1) * 128], ident8[:])
                        if jc % 2 == 1:
                            yield
                    nc.scalar.activation(
                        wtT[:, :, 0:128],
                        tps08[:, :, 0:1].rearrange("p (a b) o -> p a b o", a=NB),
                        ACTF.Copy)
                    yield
                    if DEBUG and h == 0:
                        nc.sync.dma_start(out=dbg["w0"], in_=w0[:])

                def gen_ut(h):
                    """U^T[j,t] for t>=128 via fp8 DoubleRow; exp emits W^T."""
                    d = st[h]
                    a1, sqT, wtT = d["a1"], d["sqT"], d["wtT"]
                    sqA = sqT[:].rearrange("p (b c) -> p b c", c=1024)  # blocks 0-3
                    sqB = sqT[:].rearrange("p (b c) -> p b c", c=512)   # blocks 4-7
                    for jc in range(NB):
                        jsl = slice(jc * 128, (jc + 1) * 128)
                        p1 = ppm.tile([128, 384], F32, tag="mm", name="ps_ut1")
                        nc.tensor.matmul(p1[:], a1[:, 0:2, jsl], sqA[:, 0:2, 128:512],
                                         start=True, stop=False, perf_mode=DR)
                        nc.tensor.matmul(p1[:], a1[:, 2:4, jsl], sqA[:, 2:4, 128:512],
                                         start=False, stop=True, perf_mode=DR)
                        yield
                        nc.scalar.activation(wtT[:, jc, 128:512], p1[:], ACTF.Exp,
                                             scale=1.0 / 16)
                        p2 = ppm.tile([128, 512], F32, tag="mm", name="ps_ut2")
                        nc.tensor.matmul(p2[:], a1[:, 0:2, jsl], sqA[:, 0:2, 512:1024],
                                         start=True, stop=False, perf_mode=DR)
                        nc.tensor.matmul(p2[:], a1[:, 2:4, jsl], sqA[:, 2:4, 512:1024],
                                         start=False, stop=False, perf_mode=DR)
                        nc.tensor.matmul(p2[:], a1[:, 4:6, jsl], sqB[:, 8:10, :],
                                         start=False, stop=False, perf_mode=DR)
                        nc.tensor.matmul(p2[:], a1[:, 6:8, jsl], sqB[:, 10:12, :],
                                         start=False, stop=True, perf_mode=DR)
                        yield
                        nc.scalar.activation(wtT[:, jc, 512:1024], p2[:], ACTF.Exp,
                                             scale=1.0 / 16)
                    if DEBUG and h == 0:
                        nc.sync.dma_start(
                            out=dbg["wtT"].rearrange("p (a b) -> p a b", a=NB),
                            in_=wtT[:])

                def gen_a1t(h):
                    """A1^T via PE transposes of fp8 a1 + one copy per block."""
                    d = st[h]
                    a1 = d["a1"]
                    a1T = ap.tile([128, NB, S], F8, tag="a1t", bufs=1, name="a1T")
                    d["a1T"] = a1T
                    for m in range(NB):
                        tps = ppt.tile([128, S], BF16, tag="tp", name="tps")
                        tps8 = tps[:].bitcast(F8).rearrange("p (a o) -> p a o", o=2)
                        for k in range(NB):
                            nc.tensor.transpose(
                                tps8[:, k * 128:(k + 1) * 128, 0:1],
                                a1[:, m, k * 128:(k + 1) * 128], ident8[:])
                        yield
                        src = tps8[:, :, 0:1].rearrange("p (a b) o -> p a b o", a=NB)
                        if m == 7:
                            nc.scalar.activation(
                                a1T[:, :, m * 128:(m + 1) * 128], src, ACTF.Copy)
                        else:
                            nc.vector.tensor_copy(
                                a1T[:, :, m * 128:(m + 1) * 128], src)
                        yield

                def gen_den(h):
                    """den[t] = sum_j wtT[j,t] via DoubleRow x ones (free)."""
                    d = st[h]
                    wtT = d["wtT"]
                    dps = ppm.tile([128, NB], F32, tag="mm", name="ps_den")
                    for i in range(NB):
                        for k in range(4):
                            nc.tensor.matmul(
                                dps[:, i:i + 1],
                                wtT[:, 2 * k:2 * k + 2, i * 128:(i + 1) * 128],
                                ones8[:], start=(k == 0), stop=(k == 3),
                                perf_mode=DR)
                        if i % 2 == 1:
                            yield
                    denB = sp.tile([128, NB], F32, tag="denB", name="denB")
                    nc.vector.tensor_copy(denB[:], dps[:])
                    recden = sp.tile([128, NB], F32, tag="recden", name="recden")
                    nc.vector.reciprocal(recden[:], denB[:])
                    gsc = sp.tile([128, NB], F32, tag="gsc", name="gsc")
                    nc.vector.tensor_tensor(gsc[:], recden[:], invidx[:], ALU.mult)
                    d["gsc"] = gsc
                    yield
                    if DEBUG and h == 0:
                        nc.sync.dma_start(out=dbg["den"], in_=denB[:])

                def gen_s2(h):
                    """S2^T[s,t] = sum_j A1^T[j,s] W^T[j,t] (fp8 DoubleRow),
                    tril-masked on the diagonal, stored compact bf16."""
                    d = st[h]
                    a1T, wtT = d["a1T"], d["wtT"]
                    s2c = ap.tile([128, S2TOT], BF16, tag="s2", name="s2c")
                    d["s2c"] = s2c
                    for m in range(NB):
                        msl = slice(m * 128, (m + 1) * 128)
                        if m < 4:
                            chunks = [(m * 128, 512 - m * 128), (512, 512)]
                        else:
                            chunks = [(m * 128, 1024 - m * 128)]
                        for t0, w in chunks:
                            ps = ppm.tile([128, w], F32, tag="mm", name="ps_s2")
                            for k in range(4):
                                nc.tensor.matmul(
                                    ps[:], a1T[:, 2 * k:2 * k + 2, msl],
                                    wtT[:, 2 * k:2 * k + 2, t0:t0 + w],
                                    start=(k == 0), stop=(k == 3), perf_mode=DR)
                            yield
                            base = S2BASE[m] + (t0 - m * 128)
                            if t0 == m * 128:
                                nc.vector.tensor_tensor(
                                    s2c[:, base:base + 128], ps[:, 0:128],
                                    mask[:, 0, 0:128], ALU.mult)
                                if w > 128:
                                    nc.scalar.activation(
                                        s2c[:, base + 128:base + w], ps[:, 128:w],
                                        ACTF.Copy)
                            else:
                                nc.scalar.activation(s2c[:, base:base + w], ps[:],
                                                     ACTF.Copy)
                            yield
                    if DEBUG and h == 0:
                        nc.sync.dma_start(out=dbg["s2c"], in_=s2c[:])

                oNs = {}

                def emit_o(h):
                    """O[t,d] = gsc[t] * sum_{s<=t} S2T[s,t] vh[s,d]; heads
                    pair into one oN tile; PE transpose -> oT [d,t].  For the
                    last head the transpose + output projection are pipelined
                    per t-block to shrink the tail."""
                    d = st.pop(h)
                    s2c, gsc = d["s2c"], d["gsc"]
                    if h % 2 == 0:
                        oNs[h // 2] = sp.tile([128, NB, 128], BF16, tag="oN",
                                              bufs=4, name="oN")
                    oN = oNs[h // 2]
                    d0 = (h % 2) * 64
                    last = (h == HG - 1)
                    if not last:
                        # all 8 t-blocks share one psum bank (single
                        # accumulation group via the lazy zero-region), then
                        # one DVE multiply against a Pool-built broadcast gsc
                        gse = sp.tile([128, NB, 64], F32, tag="gse", name="gse")
                        nc.gpsimd.tensor_copy(
                            gse[:], gsc[:].broadcast_to((128, NB, 64)))
                        ps = ppm.tile([128, 512], F32, tag="mm", name="ps_o")
                        for i in range(NB):
                            for m in range(i + 1):
                                nc.tensor.matmul(
                                    ps[:, i * 64:(i + 1) * 64],
                                    s2c[:, S2BASE[m] + (i - m) * 128:
                                        S2BASE[m] + (i - m) * 128 + 128],
                                    vh[:, m, h * 64:(h + 1) * 64],
                                    start=(i == 0 and m == 0),
                                    stop=(i == NB - 1 and m == i),
                                    skip_group_check=True)
                            if i % 2 == 1:
                                yield
                        nc.vector.tensor_tensor(
                            oN[:, :, d0:d0 + 64],
                            ps[:].rearrange("p (a d) -> p a d", a=NB),
                            gse[:], ALU.mult)
                        yield
                    for i in range(NB if last else 0):
                        ps = ppm.tile([128, 64], F32, tag="mm", name="ps_o")
                        for m in range(i + 1):
                            nc.tensor.matmul(
                                ps[:], s2c[:, S2BASE[m] + (i - m) * 128:
                                           S2BASE[m] + (i - m) * 128 + 128],
                                vh[:, m, h * 64:(h + 1) * 64],
                                start=(m == 0), stop=(m == i))
                        nc.vector.tensor_scalar(oN[:, i, d0:d0 + 64], ps[:],
                                                gsc[:, i:i + 1], None, ALU.mult)
                        if last:
                            tps = ppm.tile([128, 128], BF16, tag="a1ps", bufs=2,
                                           name="tpo")
                            nc.tensor.transpose(tps[:], oN[:, i, :], ident[:])
                            nc.scalar.activation(
                                oT[:, h // 2, i * 128:(i + 1) * 128], tps[:],
                                ACTF.Copy)
                            # lag the output-projection tile one block so its
                            # matmuls never stall PE on the oT copy just above
                            if i >= 1:
                                emit_final_tile(i - 1)
                        yield
                    if last:
                        emit_final_tile(NB - 1)
                    if h % 2 == 1 and not last:
                        oN = oNs.pop(h // 2)
                        tps = ppm.tile([128, S], BF16, tag="mm", name="tpo")
                        for i in range(NB):
                            nc.tensor.transpose(
                                tps[:, i * 128:(i + 1) * 128], oN[:, i, :], ident[:])
                        nc.scalar.activation(
                            oT[:, h // 2, :],
                            tps[:].rearrange("p (a b) -> p a b", a=NB), ACTF.Copy)

                def emit_final_tile(i):
                    for c in range(2):
                        ps = ppm.tile([128, 512], F32, tag="mm", name="ps_fin")
                        for g2 in range(2):
                            nc.tensor.matmul(
                                ps[:], oT[:, g2, i * 128:(i + 1) * 128],
                                wct[:, g2, c * 512:(c + 1) * 512],
                                start=(g2 == 0), stop=(g2 == 1))
                        ot = sp.tile([128, 512], BF16, tag="ot", bufs=3, name="ot")
                        nc.vector.tensor_copy(ot[:], ps[:])
                        nc.sync.dma_start(
                            out=out_d[i * 128:(i + 1) * 128, c * 512:(c + 1) * 512],
                            in_=ot[:])

                def chain(*gens):
                    for gg in gens:
                        yield from gg

                # q/k projections interleaved with head 0's A1 matmuls
                # (which need only qhT+pTt) so the PE stream is never
                # head-of-line blocked on a projection DMA
                gen0 = gen_a1_sq(0)

                def pull0(k):
                    for _ in range(k):
                        if next(gen0, "done") == "done":
                            break

                for gi, (g, n) in enumerate(((0, 0), (0, 1), (1, 0), (1, 1))):
                    ps = ppm.tile([128, 512], F32, tag="mm", name="ps_proj")
                    for kb in range(NB):
                        nc.tensor.matmul(
                            ps[:], wqt[:, kb, g * 128:(g + 1) * 128],
                            qTt[:, kb, n * 512:(n + 1) * 512],
                            start=(kb == 0), stop=(kb == NB - 1))
                    nc.scalar.activation(
                        qhT[:, g, n * 512:(n + 1) * 512], ps[:],
                        ACTF.Identity, bias=wqb[:, g:g + 1], scale=NORM_D)
                    nc.gpsimd.tensor_tensor(
                        qsT[:, g, n * 512:(n + 1) * 512],
                        qhT[:, g, n * 512:(n + 1) * 512],
                        invrow[:, n, :], ALU.mult)
                    pull0((4, 4, 2, 2)[gi])
                for g in range(2):
                    for n in range(2):
                        ps = ppm.tile([128, 512], F32, tag="mm", name="ps_projk")
                        for kb in range(NB):
                            nc.tensor.matmul(
                                ps[:], wkt[:, kb, g * 128:(g + 1) * 128],
                                kTt[:, kb, n * 512:(n + 1) * 512],
                                start=(kb == 0), stop=(kb == NB - 1))
                        nc.scalar.activation(
                            khT[:, g, n * 512:(n + 1) * 512], ps[:],
                            ACTF.Identity, bias=wkb[:, g:g + 1], scale=1.0)
                        pull0(2)

                # vh[s, d] = sum_c vT[c, s] wv[c, d] + wv_b[d], interleaved
                # with head 0's A1/SqT so PE has work while vT streams in
                for m2 in range(0, NB, 2):
                    # two s-blocks share one psum bank: the first start marks
                    # the whole 2KB zero-region, the second block accumulates
                    # into its (lazily zeroed) half -> one copy per pair
                    ps = ppm.tile([128, 2 * DL], F32, tag="mm", name="ps_vh")
                    for mo in range(2):
                        m = m2 + mo
                        psl = ps[:, mo * DL:(mo + 1) * DL]
                        for kb in range(NB):
                            nc.tensor.matmul(
                                psl, vTt[:, kb, m * 128:(m + 1) * 128],
                                wvt[:, kb, :],
                                start=(m2 == m and kb == 0 and mo == 0),
                                stop=False, skip_group_check=True)
                        nc.tensor.matmul(psl, ones1[:], wvb[:], start=False,
                                         stop=(mo == 1), skip_group_check=True)
                        pull0(3)
                    nc.scalar.activation(vh[:, m2:m2 + 2, :],
                                         ps[:].rearrange("p (a d) -> p a d", a=2),
                                         ACTF.Copy)
                for _ in gen0:
                    pass

                for h in range(HG):
                    gnext = gen_a1_sq(h + 1) if h + 1 < HG else None
                    # O(h-1) first: its inputs are long-ready, so it fills
                    # the phase-boundary bubble while head h's a1/sqT fp8
                    # chains drain
                    parts = [emit_o(h - 1)] if h >= 1 else []
                    last = h == HG - 1
                    wgen = chain(*parts, gen_chunk0(h), gen_ut(h),
                                 *([] if last else [gen_a1t(h)]), gen_den(h))
                    for wi, _ in enumerate(wgen):
                        if gnext is not None:
                            next(gnext, None)
                            if wi < 12:
                                next(gnext, None)
                    if last:
                        # tail: S2's block m only needs a1T's m-chunk, and
                        # O's t-block i only needs s2c blocks m<=i -- so a1T
                        # production, S2, O and the output projection all
                        # pipeline per-block
                        a1tg = gen_a1t(h)
                        next(a1tg, None)
                        next(a1tg, None)
                        og = emit_o(h)
                        osteps = 0
                        yields_at_block_done = [4, 8, 12, 16, 18, 20, 22, 24]
                        nyield = 0
                        for _ in gen_s2(h):
                            nyield += 1
                            next(a1tg, None)
                            blocks_done = sum(
                                1 for yy in yields_at_block_done if nyield >= yy)
                            while og is not None and osteps < blocks_done:
                                if next(og, "done") == "done":
                                    og = None
                                    break
                                osteps += 1
                        if og is not None:
                            for _ in og:
                                pass
                    else:
                        for _ in gen_s2(h):
                            if gnext is not None:
                                next(gnext, None)
                        if gnext is not None:
                            for _ in gnext:
                                pass

            jp_cm.__exit__(None, None, None)
            vp_cm.__exit__(None, None, None)

    nc.finalize()
    return nc


_CACHE = {}


def _get_program():
    if "nc" not in _CACHE:
        _CACHE["nc"] = _build_program()
    return _CACHE["nc"]


def _consts():
    if "consts" not in _CACHE:
        p_ = np.arange(128, dtype=np.float32)[:, None]
        c_ = np.arange(512, dtype=np.float32)[None, :]
        mask4 = np.stack(
            [(p_ + 128.0 * r <= c_) for r in range(4)]).astype(NPBF)
        ident = np.eye(128, dtype=np.float32).astype(NPBF)
        ident8 = np.eye(128, dtype=np.float32).astype(NPF8)
        blk = np.arange(NB, dtype=np.float32)[None, :]
        invidx = (1.0 / (blk * 128.0 + p_ + 1.0)).astype(np.float32)
        ones1 = np.ones((1, 128), NPBF)
        ones8 = np.ones((128, 2), NPF8)
        n_ = np.arange(2, dtype=np.float32)[:, None]
        c2_ = np.arange(512, dtype=np.float32)[None, :]
        invrow = np.broadcast_to(
            (16.0 / (n_ * 512.0 + c2_ + 1.0))[None, :, :],
            (128, 2, 512)).astype(NPBF)
        invrow = np.ascontiguousarray(invrow)
        _CACHE["consts"] = (mask4, ident, ident8, invidx, ones1, ones8, invrow)
    return _CACHE["consts"]


PROFILE = False
LAST_RESULTS = None


def kernel(v, k, q, p, wq_k, wq_b, wk_k, wk_b, wv_k, wv_b, wc_k, wc_b):
    global LAST_RESULTS
    nc = _get_program()
    mask4, ident, ident8, invidx, ones1, ones8, invrow = _consts()

    qT = [np.ascontiguousarray(q[b].T).astype(NPBF) for b in range(B)]
    kT = [np.ascontiguousarray(k[b].T).astype(NPBF) for b in range(B)]
    vT = [np.ascontiguousarray(v[b].T).astype(NPBF) for b in range(B)]
    pT = [np.ascontiguousarray(p[b].T).astype(NPBF) for b in range(B)]
    wqc = wq_k.astype(NPBF)
    wkc = wk_k.astype(NPBF)
    wvc = wv_k.astype(NPBF)
    wcc = wc_k.astype(NPBF)

    in_maps = []
    for c in range(8):
        b, hg = c // 4, c % 4
        c0 = hg * DL
        wqb = np.ascontiguousarray(
            (wq_b[c0:c0 + DL].reshape(2, 128).T * NORM_D).astype(np.float32))
        wkb = np.ascontiguousarray(wk_b[c0:c0 + DL].reshape(2, 128).T.astype(np.float32))
        in_maps.append({
            "qT": qT[b], "kT": kT[b], "vT": vT[b],
            "pT": np.ascontiguousarray(pT[b][c0:c0 + DL]),
            "wq": np.ascontiguousarray(wqc[:, c0:c0 + DL]),
            "wk": np.ascontiguousarray(wkc[:, c0:c0 + DL]),
            "wv": np.ascontiguousarray(wvc[:, c0:c0 + DL]),
            "wc": np.ascontiguousarray(wcc[c0:c0 + DL, :]),
            "wqb": wqb, "wkb": wkb,
            "wvb": np.ascontiguousarray(wv_b[c0:c0 + DL].reshape(1, DL).astype(NPBF)),
            "ones1": ones1, "ones8": ones8, "mask4": mask4, "ident": ident,
            "ident8": ident8, "invidx": invidx, "invrow": invrow,
        })

    res = run_bass_kernel_spmd(
        nc, in_maps, core_ids=list(range(8)), trace=PROFILE)
    LAST_RESULTS = res

    out = np.zeros((B, S, DM), np.float32)
    for c in range(8):
        out[c // 4] += res.results[c]["out"].astype(np.float32)
    out += wc_b[None, None, :].astype(np.float32)
    return out


# revision 143
# speedup vs baseline: 1.0055x; 1.0002x over previous
"""Trainium2 Bass kernel for nn_MultiHeadAttention_75548474736720.

Linear-attention-style MHA with causal prefix sums (see reference.py):
  A1 = elu(qh ph^T) + 1                     [s,j] per (b,h)
  U  = (tril(qh kh^T)/idx) @ A1 ; W = softmax_j(U)
  S2 = tril(W A1^T) ; out = (S2 @ vh)/idx reshaped @ wc + b

Sharding: 8 cores = (batch 2) x (head-group 4, 4 heads each); host sums
the row-sliced output projections per batch.

Key structure (v2, fp8 DoubleRow):
  - Projections + A1/SqT matmuls stay bf16; a1/sqT are STORED fp8(e4m3).
  - sqT carries 16*score/(t+1) (invidx folded via a pre-scaled q~hT),
    so exp(U/16) needs no per-t scale.
  - U is computed TRANSPOSED ([j,t]) with fp8 DoubleRow matmuls (2
    k-tiles per instr at 0.5 cyc/row): the exp then emits W^T directly
    -- no W transposes, no per-t scale problem.  Softmax shift
    invariance makes the +1 in A1 harmless in every route (it adds a
    j-independent per-t shift absorbed by the denominator).
  - t<128 (the only rows with large logits, ~19) go through an exact
    bf16 path: U0 = sqT0^T a1b0, per-row max subtraction (per-partition
    ACT bias), exp -> fp8 W0, PE-transposed into wtT columns.
  - den[t] = sum_j W^T via DoubleRow matmuls against a ones column
    (out free size 1 => ~free); gsc = invidx/den folds all
    normalization into the O-stage ACT scale (per-t scale invariance).
  - S2^T = a1T x wtT with fp8 DoubleRow; O/out-projection stay bf16.
"""

import sys

sys.path.insert(0, "/opt/trn_rl_repo")

import ml_dtypes
import numpy as np

import concourse.bass as bass  # noqa: F401  (registers AP machinery)
import concourse.mybir as mybir
from concourse import bacc
from concourse.tile import TileContext
from concourse.bass_utils import run_bass_kernel_spmd

F32 = mybir.dt.float32
BF16 = mybir.dt.bfloat16
F8 = mybir.dt.float8e4
ACTF = mybir.ActivationFunctionType
ALU = mybir.AluOpType
AXL = mybir.AxisListType
DR = mybir.MatmulPerfMode.DoubleRow
NPBF = ml_dtypes.bfloat16
NPF8 = ml_dtypes.float8_e4m3

B, S, DM, H = 2, 1024, 1024, 16
D = DM // H            # 64, head dim
HG = 4                 # heads per core
DL = HG * D            # 256, local dm slice
NB = S // 128          # 8 s-blocks
NORM_D = 0.125         # 1/sqrt(D)

# compact sqT layout: per s-block m, columns stored from t = 512*(m//4)
SQBASE = [0, 1024, 2048, 3072, 4096, 4608, 5120, 5632]  # total 6144
# compact s2T layout: per s-block m, columns stored from t = 128*m
S2BASE = [0, 1024, 1920, 2688, 3328, 3840, 4224, 4480]  # total 4608
S2TOT = 4608

DEBUG = False


def _sq_off(m, t0):
    return SQBASE[m] + t0 - 512 * (m // 4)


def _build_program():
    nc = bacc.Bacc(None, target_bir_lowering=False)

    qT_in = nc.declare_dram_parameter("qT", [DM, S], BF16, isOutput=False)
    kT_in = nc.declare_dram_parameter("kT", [DM, S], BF16, isOutput=False)
    vT_in = nc.declare_dram_parameter("vT", [DM, S], BF16, isOutput=False)
    pT_in = nc.declare_dram_parameter("pT", [DL, S], BF16, isOutput=False)
    wq_in = nc.declare_dram_parameter("wq", [DM, DL], BF16, isOutput=False)
    wk_in = nc.declare_dram_parameter("wk", [DM, DL], BF16, isOutput=False)
    wv_in = nc.declare_dram_parameter("wv", [DM, DL], BF16, isOutput=False)
    wc_in = nc.declare_dram_parameter("wc", [DL, S], BF16, isOutput=False)
    wqb_in = nc.declare_dram_parameter("wqb", [128, 2], F32, isOutput=False)
    wkb_in = nc.declare_dram_parameter("wkb", [128, 2], F32, isOutput=False)
    wvb_in = nc.declare_dram_parameter("wvb", [1, DL], BF16, isOutput=False)
    ones_in = nc.declare_dram_parameter("ones1", [1, 128], BF16, isOutput=False)
    ones8_in = nc.declare_dram_parameter("ones8", [128, 2], F8, isOutput=False)
    mask_in = nc.declare_dram_parameter("mask4", [4, 128, 512], BF16, isOutput=False)
    ident_in = nc.declare_dram_parameter("ident", [128, 128], BF16, isOutput=False)
    ident8_in = nc.declare_dram_parameter("ident8", [128, 128], F8, isOutput=False)
    inv_in = nc.declare_dram_parameter("invidx", [128, NB], F32, isOutput=False)
    invrow_in = nc.declare_dram_parameter("invrow", [128, 2, 512], BF16,
                                          isOutput=False)
    out_d = nc.declare_dram_parameter("out", [S, DM], BF16, isOutput=True)
    dbg = {}
    if DEBUG:
        dbg["a1"] = nc.declare_dram_parameter("d_a1", [128, NB * S], F8, isOutput=True)
        dbg["sqT"] = nc.declare_dram_parameter("d_sqT", [128, 6144], F8, isOutput=True)
        dbg["wtT"] = nc.declare_dram_parameter("d_wtT", [128, NB * S], F8, isOutput=True)
        dbg["den"] = nc.declare_dram_parameter("d_den", [128, NB], F32, isOutput=True)
        dbg["s2c"] = nc.declare_dram_parameter("d_s2c", [128, S2TOT], BF16, isOutput=True)
        dbg["w0"] = nc.declare_dram_parameter("d_w0", [128, S], F8, isOutput=True)

    with TileContext(nc) as tc:
        with tc.tile_pool(name="persist", bufs=1) as cp, \
             tc.tile_pool(name="ppm", bufs=4, space="PSUM") as ppm, \
             tc.tile_pool(name="ppt", bufs=2, space="PSUM") as ppt:

            mask = cp.tile([128, 4, 512], BF16)
            ident = cp.tile([128, 128], BF16)
            ident8 = cp.tile([128, 128], F8)
            invidx = cp.tile([128, NB], F32)
            invrow = cp.tile([128, 2, 512], BF16)
            wqb = cp.tile([128, 2], F32)
            wkb = cp.tile([128, 2], F32)
            wvb = cp.tile([1, DL], BF16)
            ones1 = cp.tile([1, 128], BF16)
            ones8 = cp.tile([128, 2, 1], F8)
            pTt = cp.tile([128, 2, S], BF16)
            qhT = cp.tile([128, 2, S], BF16)
            qsT = cp.tile([128, 2, S], BF16)   # q~ = qh * 16/(t+1)
            khT = cp.tile([128, 2, S], BF16)
            vh = cp.tile([128, NB, DL], BF16)
            oT = cp.tile([128, 2, S], BF16)
            wct = cp.tile([128, 2, S], BF16)

            # ---------------- projections ----------------
            vp_cm = tc.tile_pool(name="vproj", bufs=1)
            vp = vp_cm.__enter__()
            wvt = vp.tile([128, NB, DL], BF16)
            vTt = vp.tile([128, NB, S], BF16)
            jp_cm = tc.tile_pool(name="proj", bufs=1)
            jp = jp_cm.__enter__()
            if True:
                wqt = jp.tile([128, NB, DL], BF16)
                wkt = jp.tile([128, NB, DL], BF16)
                qTt = jp.tile([128, NB, S], BF16)
                kTt = jp.tile([128, NB, S], BF16)
                # The DMA transfer pipe is serial (exclusive DMA_ENGINES,
                # FIFO in issue order), so issue in strict priority order
                # with few, large descriptors: q first, then p (a1 needs
                # it), then k, then v/consts.
                def load_wx(wt_, wsrc, xt_, xsrc):
                    # weight whole, activations split by t-half so the first
                    # projection matmuls can start on half the data
                    nc.sync.dma_start(
                        out=wt_[:], in_=wsrc.rearrange("(a p) d -> p a d", p=128))
                    for nh in range(2):
                        nc.sync.dma_start(
                            out=xt_[:, :, nh * 512:(nh + 1) * 512],
                            in_=xsrc[:, nh * 512:(nh + 1) * 512].rearrange(
                                "(a p) t -> p a t", p=128))

                nc.sync.dma_start(
                    out=wqt[:, :, 0:128],
                    in_=wq_in[:, 0:128].rearrange("(a p) d -> p a d", p=128))
                nc.sync.dma_start(
                    out=qTt[:, :, 0:512],
                    in_=qT_in[:, 0:512].rearrange("(a p) t -> p a t", p=128))
                nc.sync.dma_start(
                    out=pTt[:], in_=pT_in.rearrange("(g p) t -> p g t", p=128))
                nc.sync.dma_start(out=wqb[:], in_=wqb_in[:])
                nc.sync.dma_start(out=invrow[:], in_=invrow_in[:])
                nc.sync.dma_start(
                    out=qTt[:, :, 512:1024],
                    in_=qT_in[:, 512:1024].rearrange("(a p) t -> p a t", p=128))
                nc.sync.dma_start(
                    out=wqt[:, :, 128:256],
                    in_=wq_in[:, 128:256].rearrange("(a p) d -> p a d", p=128))
                load_wx(wkt, wk_in, kTt, kT_in)
                nc.sync.dma_start(out=wkb[:], in_=wkb_in[:])
                nc.sync.dma_start(
                    out=mask[:], in_=mask_in.rearrange("r p c -> p r c"))
                nc.sync.dma_start(out=invidx[:], in_=inv_in[:])
                # tiny bias consts BEFORE the 2.5MB v loads: the vh matmul
                # groups end with the ones1 x wvb bias matmul, which must not
                # head-of-line block PE behind the vT transfers
                nc.sync.dma_start(out=wvb[:], in_=wvb_in[:])
                nc.sync.dma_start(out=ones1[:], in_=ones_in[:])
                load_wx(wvt, wv_in, vTt, vT_in)
                nc.sync.dma_start(out=ident8[:], in_=ident8_in[:])
                nc.sync.dma_start(
                    out=ones8[:], in_=ones8_in.rearrange("p (a o) -> p a o", o=1))
                nc.sync.dma_start(out=ident[:], in_=ident_in[:])
                nc.sync.dma_start(
                    out=wct[:], in_=wc_in.rearrange("(a p) t -> p a t", p=128))

            # ---------------- attention (4 heads) ----------------
            with tc.tile_pool(name="attn", bufs=2) as ap, \
                 tc.tile_pool(name="scr", bufs=2) as sp:
                st = {}

                def gen_a1_sq(h):
                    """a1 = elu(x)+1 (fp8), sqT = 16*score*invidx masked (fp8,
                    compact layout); bf16 copies of the s-block-0 pieces feed
                    the exact t<128 softmax path."""
                    g, p0 = h // 2, (h % 2) * 64
                    a1 = ap.tile([128, NB, S], F8, tag="a1", name="a1")
                    sqT = ap.tile([128, 6144], F8, tag="sq", name="sqT")
                    a1b0 = ap.tile([128, S], BF16, tag="a1b0", name="a1b0")
                    sqT0 = ap.tile([128, 128], BF16, tag="sq0", name="sqT0")
                    st[h] = dict(a1=a1, sqT=sqT, a1b0=a1b0, sqT0=sqT0)
                    for m in range(NB):
                        for c in range(2):
                            ps = ppm.tile([128, 512], F32, tag="a1ps", bufs=2,
                                          name="ps_a1")
                            nc.tensor.matmul(
                                ps[:], qhT[p0:p0 + 64, g, m * 128:(m + 1) * 128],
                                pTt[p0:p0 + 64, g, c * 512:(c + 1) * 512],
                                start=True, stop=True)
                            e = sp.tile([128, 512], F32, tag="e", bufs=4, name="e")
                            nc.scalar.activation(e[:], ps[:], ACTF.Exp)
                            e1 = sp.tile([128, 512], F32, tag="e1", bufs=4, name="e1")
                            nc.gpsimd.tensor_scalar_min(e1[:], e[:], 1.0)
                            if m == 0:
                                # bf16 master for the exact t<128 path; Pool
                                # (SBUF-only) derives the fp8 copy from it
                                nc.vector.scalar_tensor_tensor(
                                    a1b0[:, c * 512:(c + 1) * 512], ps[:], 0.0,
                                    e1[:], ALU.max, ALU.add)
                                nc.gpsimd.tensor_copy(
                                    a1[:, m, c * 512:(c + 1) * 512],
                                    a1b0[:, c * 512:(c + 1) * 512])
                            else:
                                nc.vector.scalar_tensor_tensor(
                                    a1[:, m, c * 512:(c + 1) * 512], ps[:], 0.0,
                                    e1[:], ALU.max, ALU.add)
                            yield
                    for m in range(NB):
                        for n in range(m // 4, 2):
                            ps = ppm.tile([128, 512], F32, tag="mm", name="ps_sq")
                            nc.tensor.matmul(
                                ps[:], khT[p0:p0 + 64, g, m * 128:(m + 1) * 128],
                                qsT[p0:p0 + 64, g, n * 512:(n + 1) * 512],
                                start=True, stop=True)
                            dst = sqT[:, _sq_off(m, n * 512):_sq_off(m, n * 512) + 512]
                            if n == m // 4:
                                nc.vector.tensor_tensor(dst, ps[:], mask[:, m % 4, :],
                                                        ALU.mult)
                                if m == 0:
                                    nc.vector.tensor_tensor(
                                        sqT0[:], ps[:, 0:128], mask[:, 0, 0:128],
                                        ALU.mult)
                            else:
                                nc.scalar.activation(dst, ps[:], ACTF.Copy)
                            yield
                    if DEBUG and h == 0:
                        nc.sync.dma_start(
                            out=dbg["a1"].rearrange("p (a b) -> p a b", a=NB), in_=a1[:])
                        nc.sync.dma_start(out=dbg["sqT"][:, :], in_=sqT[:])

                def gen_chunk0(h):
                    """Exact softmax path for t<128: bf16 U0, per-row max
                    subtraction, exp -> fp8, PE transpose into wtT cols 0:128."""
                    d = st[h]
                    wtT = ap.tile([128, NB, S], F8, tag="wt", name="wtT")
                    d["wtT"] = wtT
                    psA = ppt.tile([128, S], BF16, tag="tp",
                                   name="ps_u0a")[:].bitcast(F32)
                    nc.tensor.matmul(psA, d["sqT0"][:], d["a1b0"][:, 0:512],
                                     start=True, stop=True)
                    yield
                    psB = ppt.tile([128, S], BF16, tag="tp",
                                   name="ps_u0b")[:].bitcast(F32)
                    nc.tensor.matmul(psB, d["sqT0"][:], d["a1b0"][:, 512:1024],
                                     start=True, stop=True)
                    yield
                    mxa = sp.tile([128, 1], F32, tag="mxa", name="mxa")
                    mxb = sp.tile([128, 1], F32, tag="mxb", name="mxb")
                    nc.vector.tensor_reduce(mxa[:], psA, AXL.X, ALU.max)
                    nc.vector.tensor_reduce(mxb[:], psB, AXL.X, ALU.max)
                    bias0 = sp.tile([128, 1], F32, tag="bias0", name="bias0")
                    nc.vector.tensor_tensor(bias0[:], mxa[:], mxb[:], ALU.max)
                    # exp arg = U0/16 - mx0/16 + 5  (max logit -> e^5=148 < 240)
                    nc.vector.tensor_scalar(bias0[:], bias0[:], -1.0 / 16, 5.0,
                                            ALU.mult, ALU.add)
                    w0 = ap.tile([128, S], F8, tag="w0", name="w0")
                    nc.scalar.activation(w0[:, 0:512], psA, ACTF.Exp,
                                         bias=bias0[:, 0:1], scale=1.0 / 16)
                    nc.scalar.activation(w0[:, 512:1024], psB, ACTF.Exp,
                                         bias=bias0[:, 0:1], scale=1.0 / 16)
                    yield
                    # fp8 PE transposes write on 16-bit lanes: allocate the
                    # psum as bf16 and bitcast to an element-step-2 fp8 view
                    tps0 = ppt.tile([128, S], BF16, tag="tp", name="tps0")
                    tps08 = tps0[:].bitcast(F8).rearrange("p (a o) -> p a o", o=2)
                    for jc in range(NB):
                        nc.tensor.transpose(
                            tps08[:, jc * 128:(jc + 1) * 128, 0:1],
                            w0[:, jc * 128:(jc + 1) * 128], ident8[:])
                        if jc % 2 == 1:
                            yield
                    nc.scalar.activation(
                        wtT[:, :, 0:128],
                        tps08[:, :, 0:1].rearrange("p (a b) o -> p a b o", a=NB),
                        ACTF.Copy)
                    yield
                    if DEBUG and h == 0:
                        nc.sync.dma_start(out=dbg["w0"], in_=w0[:])

                def gen_ut(h):
                    """U^T[j,t] for t>=128 via fp8 DoubleRow; exp emits W^T."""
                    d = st[h]
                    a1, sqT, wtT = d["a1"], d["sqT"], d["wtT"]
                    sqA = sqT[:].rearrange("p (b c) -> p b c", c=1024)  # blocks 0-3
                    sqB = sqT[:].rearrange("p (b c) -> p b c", c=512)   # blocks 4-7
                    for jc in range(NB):
                        jsl = slice(jc * 128, (jc + 1) * 128)
                        p1 = ppm.tile([128, 384], F32, tag="mm", name="ps_ut1")
                        nc.tensor.matmul(p1[:], a1[:, 0:2, jsl], sqA[:, 0:2, 128:512],
                                         start=True, stop=False, perf_mode=DR)
                        nc.tensor.matmul(p1[:], a1[:, 2:4, jsl], sqA[:, 2:4, 128:512],
                                         start=False, stop=True, perf_mode=DR)
                        yield
                        nc.scalar.activation(wtT[:, jc, 128:512], p1[:], ACTF.Exp,
                                             scale=1.0 / 16)
                        p2 = ppm.tile([128, 512], F32, tag="mm", name="ps_ut2")
                        nc.tensor.matmul(p2[:], a1[:, 0:2, jsl], sqA[:, 0:2, 512:1024],
                                         start=True, stop=False, perf_mode=DR)
                        nc.tensor.matmul(p2[:], a1[:, 2:4, jsl], sqA[:, 2:4, 512:1024],
                                         start=False, stop=False, perf_mode=DR)
                        nc.tensor.matmul(p2[:], a1[:, 4:6, jsl], sqB[:, 8:10, :],
                                         start=False, stop=False, perf_mode=DR)
                        nc.tensor.matmul(p2[:], a1[:, 6:8, jsl], sqB[:, 10:12, :],
                                         start=False, stop=True, perf_mode=DR)
                        yield
                        nc.scalar.activation(wtT[:, jc, 512:1024], p2[:], ACTF.Exp,
                                             scale=1.0 / 16)
                    if DEBUG and h == 0:
                        nc.sync.dma_start(
                            out=dbg["wtT"].rearrange("p (a b) -> p a b", a=NB),
                            in_=wtT[:])

                def gen_a1t(h):
                    """A1^T via PE transposes of fp8 a1 + one copy per block."""
                    d = st[h]
                    a1 = d["a1"]
                    a1T = ap.tile([128, NB, S], F8, tag="a1t", bufs=1, name="a1T")
                    d["a1T"] = a1T
                    for m in range(NB):
                        tps = ppt.tile([128, S], BF16, tag="tp", name="tps")
                        tps8 = tps[:].bitcast(F8).rearrange("p (a o) -> p a o", o=2)
                        for k in range(NB):
                            nc.tensor.transpose(
                                tps8[:, k * 128:(k + 1) * 128, 0:1],
                                a1[:, m, k * 128:(k + 1) * 128], ident8[:])
                        yield
                        src = tps8[:, :, 0:1].rearrange("p (a b) o -> p a b o", a=NB)
                        if m == 7:
                            nc.scalar.activation(
                                a1T[:, :, m * 128:(m + 1) * 128], src, ACTF.Copy)
                        else:
                            nc.vector.tensor_copy(
                                a1T[:, :, m * 128:(m + 1) * 128], src)
                        yield

                def gen_den(h):
                    """den[t] = sum_j wtT[j,t] via DoubleRow x ones (free)."""
                    d = st[h]
                    wtT = d["wtT"]
                    dps = ppm.tile([128, NB], F32, tag="mm", name="ps_den")
                    for i in range(NB):
                        for k in range(4):
                            nc.tensor.matmul(
                                dps[:, i:i + 1],
                                wtT[:, 2 * k:2 * k + 2, i * 128:(i + 1) * 128],
                                ones8[:], start=(k == 0), stop=(k == 3),
                                perf_mode=DR)
                        if i % 2 == 1:
                            yield
                    denB = sp.tile([128, NB], F32, tag="denB", name="denB")
                    nc.vector.tensor_copy(denB[:], dps[:])
                    recden = sp.tile([128, NB], F32, tag="recden", name="recden")
                    nc.vector.reciprocal(recden[:], denB[:])
                    gsc = sp.tile([128, NB], F32, tag="gsc", name="gsc")
                    nc.vector.tensor_tensor(gsc[:], recden[:], invidx[:], ALU.mult)
                    d["gsc"] = gsc
                    yield
                    if DEBUG and h == 0:
                        nc.sync.dma_start(out=dbg["den"], in_=denB[:])

                def gen_s2(h):
                    """S2^T[s,t] = sum_j A1^T[j,s] W^T[j,t] (fp8 DoubleRow),
                    tril-masked on the diagonal, stored compact bf16."""
                    d = st[h]
                    a1T, wtT = d["a1T"], d["wtT"]
                    s2c = ap.tile([128, S2TOT], BF16, tag="s2", name="s2c")
                    d["s2c"] = s2c
                    for m in range(NB):
                        msl = slice(m * 128, (m + 1) * 128)
                        if m < 4:
                            chunks = [(m * 128, 512 - m * 128), (512, 512)]
                        else:
                            chunks = [(m * 128, 1024 - m * 128)]
                        for t0, w in chunks:
                            ps = ppm.tile([128, w], F32, tag="mm", name="ps_s2")
                            for k in range(4):
                                nc.tensor.matmul(
                                    ps[:], a1T[:, 2 * k:2 * k + 2, msl],
                                    wtT[:, 2 * k:2 * k + 2, t0:t0 + w],
                                    start=(k == 0), stop=(k == 3), perf_mode=DR)
                            yield
                            base = S2BASE[m] + (t0 - m * 128)
                            if t0 == m * 128:
                                nc.vector.tensor_tensor(
                                    s2c[:, base:base + 128], ps[:, 0:128],
                                    mask[:, 0, 0:128], ALU.mult)
                                if w > 128:
                                    nc.scalar.activation(
                                        s2c[:, base + 128:base + w], ps[:, 128:w],
                                        ACTF.Copy)
                            else:
                                nc.scalar.activation(s2c[:, base:base + w], ps[:],
                                                     ACTF.Copy)
                            yield
                    if DEBUG and h == 0:
                        nc.sync.dma_start(out=dbg["s2c"], in_=s2c[:])

                oNs = {}

                def emit_o(h):
                    """O[t,d] = gsc[t] * sum_{s<=t} S2T[s,t] vh[s,d]; heads
                    pair into one oN tile; PE transpose -> oT [d,t].  For the
                    last head the transpose + output projection are pipelined
                    per t-block to shrink the tail."""
                    d = st.pop(h)
                    s2c, gsc = d["s2c"], d["gsc"]
                    if h % 2 == 0:
                        oNs[h // 2] = sp.tile([128, NB, 128], BF16, tag="oN",
                                              bufs=4, name="oN")
                    oN = oNs[h // 2]
                    d0 = (h % 2) * 64
                    last = (h == HG - 1)
                    if not last:
                        # all 8 t-blocks share one psum bank (single
                        # accumulation group via the lazy zero-region), then
                        # one DVE multiply against a Pool-built broadcast gsc
                        gse = sp.tile([128, NB, 64], F32, tag="gse", name="gse")
                        nc.gpsimd.tensor_copy(
                            gse[:], gsc[:].broadcast_to((128, NB, 64)))
                        ps = ppm.tile([128, 512], F32, tag="mm", name="ps_o")
                        for i in range(NB):
                            for m in range(i + 1):
                                nc.tensor.matmul(
                                    ps[:, i * 64:(i + 1) * 64],
                                    s2c[:, S2BASE[m] + (i - m) * 128:
                                        S2BASE[m] + (i - m) * 128 + 128],
                                    vh[:, m, h * 64:(h + 1) * 64],
                                    start=(i == 0 and m == 0),
                                    stop=(i == NB - 1 and m == i),
                                    skip_group_check=True)
                            if i % 2 == 1:
                                yield
                        nc.vector.tensor_tensor(
                            oN[:, :, d0:d0 + 64],
                            ps[:].rearrange("p (a d) -> p a d", a=NB),
                            gse[:], ALU.mult)
                        yield
                    for i in range(NB if last else 0):
                        ps = ppm.tile([128, 64], F32, tag="mm", name="ps_o")
                        for m in range(i + 1):
                            nc.tensor.matmul(
                                ps[:], s2c[:, S2BASE[m] + (i - m) * 128:
                                           S2BASE[m] + (i - m) * 128 + 128],
                                vh[:, m, h * 64:(h + 1) * 64],
                                start=(m == 0), stop=(m == i))
                        nc.vector.tensor_scalar(oN[:, i, d0:d0 + 64], ps[:],
                                                gsc[:, i:i + 1], None, ALU.mult)
                        if last:
                            tps = ppm.tile([128, 128], BF16, tag="a1ps", bufs=2,
                                           name="tpo")
                            nc.tensor.transpose(tps[:], oN[:, i, :], ident[:])
                            nc.scalar.activation(
                                oT[:, h // 2, i * 128:(i + 1) * 128], tps[:],
                                ACTF.Copy)
                            # lag the output-projection tile one block so its
                            # matmuls never stall PE on the oT copy just above
                            if i >= 1:
                                emit_final_tile(i - 1)
                        yield
                    if last:
                        emit_final_tile(NB - 1)
                    if h % 2 == 1 and not last:
                        oN = oNs.pop(h // 2)
                        tps = ppm.tile([128, S], BF16, tag="mm", name="tpo")
                        for i in range(NB):
                            nc.tensor.transpose(
                                tps[:, i * 128:(i + 1) * 128], oN[:, i, :], ident[:])
                        nc.scalar.activation(
                            oT[:, h // 2, :],
                            tps[:].rearrange("p (a b) -> p a b", a=NB), ACTF.Copy)

                def emit_final_tile(i):
                    for c in range(2):
                        ps = ppm.tile([128, 512], F32, tag="mm", name="ps_fin")
                        for g2 in range(2):
                            nc.tensor.matmul(
                                ps[:], oT[:, g2, i * 128:(i + 1) * 128],
                                wct[:, g2, c * 512:(c + 1) * 512],
                                start=(g2 == 0), stop=(g2 == 1))
                        ot = sp.tile([128, 512], BF16, tag="ot", bufs=3, name="ot")
                        nc.vector.tensor_copy(ot[:], ps[:])
                        nc.sync.dma_start(
                            out=out_d[i * 128:(i + 1) * 128, c * 512:(c + 1) * 512],
                            in_=ot[:])

                def chain(*gens):
                    for gg in gens:
                        yield from gg

                # q/k projections interleaved with head 0's A1 matmuls
                # (which need only qhT+pTt) so the PE stream is never
                # head-of-line blocked on a projection DMA
                gen0 = gen_a1_sq(0)

                def pull0(k):
                    for _ in range(k):
                        if next(gen0, "done") == "done":
                            break

                for gi, (g, n) in enumerate(((0, 0), (0, 1), (1, 0), (1, 1))):
                    ps = ppm.tile([128, 512], F32, tag="mm", name="ps_proj")
                    for kb in range(NB):
                        nc.tensor.matmul(
                            ps[:], wqt[:, kb, g * 128:(g + 1) * 128],
                            qTt[:, kb, n * 512:(n + 1) * 512],
                            start=(kb == 0), stop=(kb == NB - 1))
                    nc.scalar.activation(
                        qhT[:, g, n * 512:(n + 1) * 512], ps[:],
                        ACTF.Identity, bias=wqb[:, g:g + 1], scale=NORM_D)
                    nc.gpsimd.tensor_tensor(
                        qsT[:, g, n * 512:(n + 1) * 512],
                        qhT[:, g, n * 512:(n + 1) * 512],
                        invrow[:, n, :], ALU.mult)
                    pull0((4, 4, 2, 2)[gi])
                for g in range(2):
                    for n in range(2):
                        ps = ppm.tile([128, 512], F32, tag="mm", name="ps_projk")
                        for kb in range(NB):
                            nc.tensor.matmul(
                                ps[:], wkt[:, kb, g * 128:(g + 1) * 128],
                                kTt[:, kb, n * 512:(n + 1) * 512],
                                start=(kb == 0), stop=(kb == NB - 1))
                        nc.scalar.activation(
                            khT[:, g, n * 512:(n + 1) * 512], ps[:],
                            ACTF.Identity, bias=wkb[:, g:g + 1], scale=1.0)
                        pull0(2)

                # vh[s, d] = sum_c vT[c, s] wv[c, d] + wv_b[d], interleaved
                # with head 0's A1/SqT so PE has work while vT streams in
                for m2 in range(0, NB, 2):
                    # two s-blocks share one psum bank: the first start marks
                    # the whole 2KB zero-region, the second block accumulates
                    # into its (lazily zeroed) half -> one copy per pair
                    ps = ppm.tile([128, 2 * DL], F32, tag="mm", name="ps_vh")
                    for mo in range(2):
                        m = m2 + mo
                        psl = ps[:, mo * DL:(mo + 1) * DL]
                        for kb in range(NB):
                            nc.tensor.matmul(
                                psl, vTt[:, kb, m * 128:(m + 1) * 128],
                                wvt[:, kb, :],
                                start=(m2 == m and kb == 0 and mo == 0),
                                stop=False, skip_group_check=True)
                        nc.tensor.matmul(psl, ones1[:], wvb[:], start=False,
                                         stop=(mo == 1), skip_group_check=True)
                        pull0(3)
                    nc.scalar.activation(vh[:, m2:m2 + 2, :],
                                         ps[:].rearrange("p (a d) -> p a d", a=2),
                                         ACTF.Copy)
                for _ in gen0:
                    pass

                for h in range(HG):
                    gnext = gen_a1_sq(h + 1) if h + 1 < HG else None
                    # O(h-1) first: its inputs are long-ready, so it fills
                    # the phase-boundary bubble while head h's a1/sqT fp8
                    # chains drain
                    parts = [emit_o(h - 1)] if h >= 1 else []
                    last = h == HG - 1
                    wgen = chain(*parts, gen_chunk0(h), gen_ut(h),
                                 *([] if last else [gen_a1t(h)]), gen_den(h))
                    for wi, _ in enumerate(wgen):
                        if gnext is not None:
                            next(gnext, None)
                            if wi < 12:
                                next(gnext, None)
                            if 6 <= wi < 10:
                                next(gnext, None)
                    if last:
                        # tail: S2's block m only needs a1T's m-chunk, and
                        # O's t-block i only needs s2c blocks m<=i -- so a1T
                        # production, S2, O and the output projection all
                        # pipeline per-block
                        a1tg = gen_a1t(h)
                        next(a1tg, None)
                        next(a1tg, None)
                        og = emit_o(h)
                        osteps = 0
                        yields_at_block_done = [4, 8, 12, 16, 18, 20, 22, 24]
                        nyield = 0
                        for _ in gen_s2(h):
                            nyield += 1
                            next(a1tg, None)
                            blocks_done = sum(
                                1 for yy in yields_at_block_done if nyield >= yy)
                            while og is not None and osteps < blocks_done:
                                if next(og, "done") == "done":
                                    og = None
                                    break
                                osteps += 1
                        if og is not None:
                            for _ in og:
                                pass
                    else:
                        for _ in gen_s2(h):
                            if gnext is not None:
                                next(gnext, None)
                        if gnext is not None:
                            for _ in gnext:
                                pass

            jp_cm.__exit__(None, None, None)
            vp_cm.__exit__(None, None, None)

    nc.finalize()
    return nc


_CACHE = {}


def _get_program():
    if "nc" not in _CACHE:
        _CACHE["nc"] = _build_program()
    return _CACHE["nc"]


def _consts():
    if "consts" not in _CACHE:
        p_ = np.arange(128, dtype=np.float32)[:, None]
        c_ = np.arange(512, dtype=np.float32)[None, :]
        mask4 = np.stack(
            [(p_ + 128.0 * r <= c_) for r in range(4)]).astype(NPBF)
        ident = np.eye(128, dtype=np.float32).astype(NPBF)
        ident8 = np.eye(128, dtype=np.float32).astype(NPF8)
        blk = np.arange(NB, dtype=np.float32)[None, :]
        invidx = (1.0 / (blk * 128.0 + p_ + 1.0)).astype(np.float32)
        ones1 = np.ones((1, 128), NPBF)
        ones8 = np.ones((128, 2), NPF8)
        n_ = np.arange(2, dtype=np.float32)[:, None]
        c2_ = np.arange(512, dtype=np.float32)[None, :]
        invrow = np.broadcast_to(
            (16.0 / (n_ * 512.0 + c2_ + 1.0))[None, :, :],
            (128, 2, 512)).astype(NPBF)
        invrow = np.ascontiguousarray(invrow)
        _CACHE["consts"] = (mask4, ident, ident8, invidx, ones1, ones8, invrow)
    return _CACHE["consts"]


PROFILE = False
LAST_RESULTS = None


def kernel(v, k, q, p, wq_k, wq_b, wk_k, wk_b, wv_k, wv_b, wc_k, wc_b):
    global LAST_RESULTS
    nc = _get_program()
    mask4, ident, ident8, invidx, ones1, ones8, invrow = _consts()

    qT = [np.ascontiguousarray(q[b].T).astype(NPBF) for b in range(B)]
    kT = [np.ascontiguousarray(k[b].T).astype(NPBF) for b in range(B)]
    vT = [np.ascontiguousarray(v[b].T).astype(NPBF) for b in range(B)]
    pT = [np.ascontiguousarray(p[b].T).astype(NPBF) for b in range(B)]
    wqc = wq_k.astype(NPBF)
    wkc = wk_k.astype(NPBF)
    wvc = wv_k.astype(NPBF)
    wcc = wc_k.astype(NPBF)

    in_maps = []
    for c in range(8):
        b, hg = c // 4, c % 4
        c0 = hg * DL
        wqb = np.ascontiguousarray(
            (wq_b[c0:c0 + DL].reshape(2, 128).T * NORM_D).astype(np.float32))
        wkb = np.ascontiguousarray(wk_b[c0:c0 + DL].reshape(2, 128).T.astype(np.float32))
        in_maps.append({
            "qT": qT[b], "kT": kT[b], "vT": vT[b],
            "pT": np.ascontiguousarray(pT[b][c0:c0 + DL]),
            "wq": np.ascontiguousarray(wqc[:, c0:c0 + DL]),
            "wk": np.ascontiguousarray(wkc[:, c0:c0 + DL]),
            "wv": np.ascontiguousarray(wvc[:, c0:c0 + DL]),
            "wc": np.ascontiguousarray(wcc[c0:c0 + DL, :]),
            "wqb": wqb, "wkb": wkb,
            "wvb": np.ascontiguousarray(wv_b[c0:c0 + DL].reshape(1, DL).astype(NPBF)),
            "ones1": ones1, "ones8": ones8, "mask4": mask4, "ident": ident,
            "ident8": ident8, "invidx": invidx, "invrow": invrow,
        })

    res = run_bass_kernel_spmd(
        nc, in_maps, core_ids=list(range(8)), trace=PROFILE)
    LAST_RESULTS = res

    out = np.zeros((B, S, DM), np.float32)
    for c in range(8):
        out[c // 4] += res.results[c]["out"].astype(np.float32)
    out += wc_b[None, None, :].astype(np.float32)
    return out


# revision 150
# speedup vs baseline: 1.0058x; 1.0004x over previous
"""Trainium2 Bass kernel for nn_MultiHeadAttention_75548474736720.

Linear-attention-style MHA with causal prefix sums (see reference.py):
  A1 = elu(qh ph^T) + 1                     [s,j] per (b,h)
  U  = (tril(qh kh^T)/idx) @ A1 ; W = softmax_j(U)
  S2 = tril(W A1^T) ; out = (S2 @ vh)/idx reshaped @ wc + b

Sharding: 8 cores = (batch 2) x (head-group 4, 4 heads each); host sums
the row-sliced output projections per batch.

Key structure (v2, fp8 DoubleRow):
  - Projections + A1/SqT matmuls stay bf16; a1/sqT are STORED fp8(e4m3).
  - sqT carries 16*score/(t+1) (invidx folded via a pre-scaled q~hT),
    so exp(U/16) needs no per-t scale.
  - U is computed TRANSPOSED ([j,t]) with fp8 DoubleRow matmuls (2
    k-tiles per instr at 0.5 cyc/row): the exp then emits W^T directly
    -- no W transposes, no per-t scale problem.  Softmax shift
    invariance makes the +1 in A1 harmless in every route (it adds a
    j-independent per-t shift absorbed by the denominator).
  - t<128 (the only rows with large logits, ~19) go through an exact
    bf16 path: U0 = sqT0^T a1b0, per-row max subtraction (per-partition
    ACT bias), exp -> fp8 W0, PE-transposed into wtT columns.
  - den[t] = sum_j W^T via DoubleRow matmuls against a ones column
    (out free size 1 => ~free); gsc = invidx/den folds all
    normalization into the O-stage ACT scale (per-t scale invariance).
  - S2^T = a1T x wtT with fp8 DoubleRow; O/out-projection stay bf16.
"""

import sys

sys.path.insert(0, "/opt/trn_rl_repo")

import ml_dtypes
import numpy as np

import concourse.bass as bass  # noqa: F401  (registers AP machinery)
import concourse.mybir as mybir
from concourse import bacc
from concourse.tile import TileContext
from concourse.bass_utils import run_bass_kernel_spmd

F32 = mybir.dt.float32
BF16 = mybir.dt.bfloat16
F8 = mybir.dt.float8e4
ACTF = mybir.ActivationFunctionType
ALU = mybir.AluOpType
AXL = mybir.AxisListType
DR = mybir.MatmulPerfMode.DoubleRow
NPBF = ml_dtypes.bfloat16
NPF8 = ml_dtypes.float8_e4m3

B, S, DM, H = 2, 1024, 1024, 16
D = DM // H            # 64, head dim
HG = 4                 # heads per core
DL = HG * D            # 256, local dm slice
NB = S // 128          # 8 s-blocks
NORM_D = 0.125         # 1/sqrt(D)

# compact sqT layout: per s-block m, columns stored from t = 512*(m//4)
SQBASE = [0, 1024, 2048, 3072, 4096, 4608, 5120, 5632]  # total 6144
# compact s2T layout: per s-block m, columns stored from t = 128*m
S2BASE = [0, 1024, 1920, 2688, 3328, 3840, 4224, 4480]  # total 4608
S2TOT = 4608

DEBUG = False


def _sq_off(m, t0):
    return SQBASE[m] + t0 - 512 * (m // 4)


def _build_program():
    nc = bacc.Bacc(None, target_bir_lowering=False)

    qT_in = nc.declare_dram_parameter("qT", [DM, S], BF16, isOutput=False)
    kT_in = nc.declare_dram_parameter("kT", [DM, S], BF16, isOutput=False)
    vT_in = nc.declare_dram_parameter("vT", [DM, S], BF16, isOutput=False)
    pT_in = nc.declare_dram_parameter("pT", [DL, S], BF16, isOutput=False)
    wq_in = nc.declare_dram_parameter("wq", [DM, DL], BF16, isOutput=False)
    wk_in = nc.declare_dram_parameter("wk", [DM, DL], BF16, isOutput=False)
    wv_in = nc.declare_dram_parameter("wv", [DM, DL], BF16, isOutput=False)
    wc_in = nc.declare_dram_parameter("wc", [DL, S], BF16, isOutput=False)
    wqb_in = nc.declare_dram_parameter("wqb", [128, 2], F32, isOutput=False)
    wkb_in = nc.declare_dram_parameter("wkb", [128, 2], F32, isOutput=False)
    wvb_in = nc.declare_dram_parameter("wvb", [1, DL], BF16, isOutput=False)
    ones_in = nc.declare_dram_parameter("ones1", [1, 128], BF16, isOutput=False)
    ones8_in = nc.declare_dram_parameter("ones8", [128, 2], F8, isOutput=False)
    mask_in = nc.declare_dram_parameter("mask4", [4, 128, 512], BF16, isOutput=False)
    ident_in = nc.declare_dram_parameter("ident", [128, 128], BF16, isOutput=False)
    ident8_in = nc.declare_dram_parameter("ident8", [128, 128], F8, isOutput=False)
    inv_in = nc.declare_dram_parameter("invidx", [128, NB], F32, isOutput=False)
    invrow_in = nc.declare_dram_parameter("invrow", [128, 2, 512], BF16,
                                          isOutput=False)
    out_d = nc.declare_dram_parameter("out", [S, DM], BF16, isOutput=True)
    dbg = {}
    if DEBUG:
        dbg["a1"] = nc.declare_dram_parameter("d_a1", [128, NB * S], F8, isOutput=True)
        dbg["sqT"] = nc.declare_dram_parameter("d_sqT", [128, 6144], F8, isOutput=True)
        dbg["wtT"] = nc.declare_dram_parameter("d_wtT", [128, NB * S], F8, isOutput=True)
        dbg["den"] = nc.declare_dram_parameter("d_den", [128, NB], F32, isOutput=True)
        dbg["s2c"] = nc.declare_dram_parameter("d_s2c", [128, S2TOT], BF16, isOutput=True)
        dbg["w0"] = nc.declare_dram_parameter("d_w0", [128, S], F8, isOutput=True)

    with TileContext(nc) as tc:
        with tc.tile_pool(name="persist", bufs=1) as cp, \
             tc.tile_pool(name="ppm", bufs=4, space="PSUM") as ppm, \
             tc.tile_pool(name="ppt", bufs=2, space="PSUM") as ppt:

            mask = cp.tile([128, 4, 512], BF16)
            ident = cp.tile([128, 128], BF16)
            ident8 = cp.tile([128, 128], F8)
            invidx = cp.tile([128, NB], F32)
            invrow = cp.tile([128, 2, 512], BF16)
            wqb = cp.tile([128, 2], F32)
            wkb = cp.tile([128, 2], F32)
            wvb = cp.tile([1, DL], BF16)
            ones1 = cp.tile([1, 128], BF16)
            ones8 = cp.tile([128, 2, 1], F8)
            pTt = cp.tile([128, 2, S], BF16)
            qhT = cp.tile([128, 2, S], BF16)
            qsT = cp.tile([128, 2, S], BF16)   # q~ = qh * 16/(t+1)
            khT = cp.tile([128, 2, S], BF16)
            vh = cp.tile([128, NB, DL], BF16)
            oT = cp.tile([128, 2, S], BF16)
            wct = cp.tile([128, 2, S], BF16)

            # ---------------- projections ----------------
            vp_cm = tc.tile_pool(name="vproj", bufs=1)
            vp = vp_cm.__enter__()
            wvt = vp.tile([128, NB, DL], BF16)
            vTt = vp.tile([128, NB, S], BF16)
            jp_cm = tc.tile_pool(name="proj", bufs=1)
            jp = jp_cm.__enter__()
            if True:
                wqt = jp.tile([128, NB, DL], BF16)
                wkt = jp.tile([128, NB, DL], BF16)
                qTt = jp.tile([128, NB, S], BF16)
                kTt = jp.tile([128, NB, S], BF16)
                # The DMA transfer pipe is serial (exclusive DMA_ENGINES,
                # FIFO in issue order), so issue in strict priority order
                # with few, large descriptors: q first, then p (a1 needs
                # it), then k, then v/consts.
                def load_wx(wt_, wsrc, xt_, xsrc):
                    # weight whole, activations split by t-half so the first
                    # projection matmuls can start on half the data
                    nc.sync.dma_start(
                        out=wt_[:], in_=wsrc.rearrange("(a p) d -> p a d", p=128))
                    for nh in range(2):
                        nc.sync.dma_start(
                            out=xt_[:, :, nh * 512:(nh + 1) * 512],
                            in_=xsrc[:, nh * 512:(nh + 1) * 512].rearrange(
                                "(a p) t -> p a t", p=128))

                nc.sync.dma_start(
                    out=wqt[:, :, 0:128],
                    in_=wq_in[:, 0:128].rearrange("(a p) d -> p a d", p=128))
                nc.sync.dma_start(
                    out=qTt[:, :, 0:512],
                    in_=qT_in[:, 0:512].rearrange("(a p) t -> p a t", p=128))
                nc.sync.dma_start(
                    out=pTt[:], in_=pT_in.rearrange("(g p) t -> p g t", p=128))
                nc.sync.dma_start(out=wqb[:], in_=wqb_in[:])
                nc.sync.dma_start(out=invrow[:], in_=invrow_in[:])
                nc.sync.dma_start(
                    out=qTt[:, :, 512:1024],
                    in_=qT_in[:, 512:1024].rearrange("(a p) t -> p a t", p=128))
                nc.sync.dma_start(
                    out=wqt[:, :, 128:256],
                    in_=wq_in[:, 128:256].rearrange("(a p) d -> p a d", p=128))
                load_wx(wkt, wk_in, kTt, kT_in)
                nc.sync.dma_start(out=wkb[:], in_=wkb_in[:])
                nc.sync.dma_start(
                    out=mask[:], in_=mask_in.rearrange("r p c -> p r c"))
                nc.sync.dma_start(out=invidx[:], in_=inv_in[:])
                # tiny bias consts BEFORE the 2.5MB v loads: the vh matmul
                # groups end with the ones1 x wvb bias matmul, which must not
                # head-of-line block PE behind the vT transfers
                nc.sync.dma_start(out=wvb[:], in_=wvb_in[:])
                nc.sync.dma_start(out=ones1[:], in_=ones_in[:])
                load_wx(wvt, wv_in, vTt, vT_in)
                nc.sync.dma_start(out=ident8[:], in_=ident8_in[:])
                nc.sync.dma_start(
                    out=ones8[:], in_=ones8_in.rearrange("p (a o) -> p a o", o=1))
                nc.sync.dma_start(out=ident[:], in_=ident_in[:])
                nc.sync.dma_start(
                    out=wct[:], in_=wc_in.rearrange("(a p) t -> p a t", p=128))

            # ---------------- attention (4 heads) ----------------
            with tc.tile_pool(name="attn", bufs=2) as ap, \
                 tc.tile_pool(name="scr", bufs=2) as sp:
                st = {}

                def gen_a1_sq(h):
                    """a1 = elu(x)+1 (fp8), sqT = 16*score*invidx masked (fp8,
                    compact layout); bf16 copies of the s-block-0 pieces feed
                    the exact t<128 softmax path."""
                    g, p0 = h // 2, (h % 2) * 64
                    a1 = ap.tile([128, NB, S], F8, tag="a1", name="a1")
                    sqT = ap.tile([128, 6144], F8, tag="sq", name="sqT")
                    a1b0 = ap.tile([128, S], BF16, tag="a1b0", name="a1b0")
                    sqT0 = ap.tile([128, 128], BF16, tag="sq0", name="sqT0")
                    st[h] = dict(a1=a1, sqT=sqT, a1b0=a1b0, sqT0=sqT0)
                    for m in range(NB):
                        for c in range(2):
                            ps = ppm.tile([128, 512], F32, tag="a1ps", bufs=2,
                                          name="ps_a1")
                            nc.tensor.matmul(
                                ps[:], qhT[p0:p0 + 64, g, m * 128:(m + 1) * 128],
                                pTt[p0:p0 + 64, g, c * 512:(c + 1) * 512],
                                start=True, stop=True)
                            e = sp.tile([128, 512], F32, tag="e", bufs=5, name="e")
                            nc.scalar.activation(e[:], ps[:], ACTF.Exp)
                            e1 = sp.tile([128, 512], F32, tag="e1", bufs=5, name="e1")
                            nc.gpsimd.tensor_scalar_min(e1[:], e[:], 1.0)
                            if m == 0:
                                # bf16 master for the exact t<128 path; Pool
                                # (SBUF-only) derives the fp8 copy from it
                                nc.vector.scalar_tensor_tensor(
                                    a1b0[:, c * 512:(c + 1) * 512], ps[:], 0.0,
                                    e1[:], ALU.max, ALU.add)
                                nc.gpsimd.tensor_copy(
                                    a1[:, m, c * 512:(c + 1) * 512],
                                    a1b0[:, c * 512:(c + 1) * 512])
                            else:
                                nc.vector.scalar_tensor_tensor(
                                    a1[:, m, c * 512:(c + 1) * 512], ps[:], 0.0,
                                    e1[:], ALU.max, ALU.add)
                            yield
                    for m in range(NB):
                        for n in range(m // 4, 2):
                            ps = ppm.tile([128, 512], F32, tag="mm", name="ps_sq")
                            nc.tensor.matmul(
                                ps[:], khT[p0:p0 + 64, g, m * 128:(m + 1) * 128],
                                qsT[p0:p0 + 64, g, n * 512:(n + 1) * 512],
                                start=True, stop=True)
                            dst = sqT[:, _sq_off(m, n * 512):_sq_off(m, n * 512) + 512]
                            if n == m // 4:
                                nc.vector.tensor_tensor(dst, ps[:], mask[:, m % 4, :],
                                                        ALU.mult)
                                if m == 0:
                                    nc.vector.tensor_tensor(
                                        sqT0[:], ps[:, 0:128], mask[:, 0, 0:128],
                                        ALU.mult)
                            else:
                                nc.scalar.activation(dst, ps[:], ACTF.Copy)
                            yield
                    if DEBUG and h == 0:
                        nc.sync.dma_start(
                            out=dbg["a1"].rearrange("p (a b) -> p a b", a=NB), in_=a1[:])
                        nc.sync.dma_start(out=dbg["sqT"][:, :], in_=sqT[:])

                def gen_chunk0(h):
                    """Exact softmax path for t<128: bf16 U0, per-row max
                    subtraction, exp -> fp8, PE transpose into wtT cols 0:128."""
                    d = st[h]
                    wtT = ap.tile([128, NB, S], F8, tag="wt", name="wtT")
                    d["wtT"] = wtT
                    psA = ppt.tile([128, S], BF16, tag="tp",
                                   name="ps_u0a")[:].bitcast(F32)
                    nc.tensor.matmul(psA, d["sqT0"][:], d["a1b0"][:, 0:512],
                                     start=True, stop=True)
                    yield
                    psB = ppt.tile([128, S], BF16, tag="tp",
                                   name="ps_u0b")[:].bitcast(F32)
                    nc.tensor.matmul(psB, d["sqT0"][:], d["a1b0"][:, 512:1024],
                                     start=True, stop=True)
                    yield
                    mxa = sp.tile([128, 1], F32, tag="mxa", name="mxa")
                    mxb = sp.tile([128, 1], F32, tag="mxb", name="mxb")
                    nc.vector.tensor_reduce(mxa[:], psA, AXL.X, ALU.max)
                    nc.vector.tensor_reduce(mxb[:], psB, AXL.X, ALU.max)
                    bias0 = sp.tile([128, 1], F32, tag="bias0", name="bias0")
                    nc.vector.tensor_tensor(bias0[:], mxa[:], mxb[:], ALU.max)
                    # exp arg = U0/16 - mx0/16 + 5  (max logit -> e^5=148 < 240)
                    nc.vector.tensor_scalar(bias0[:], bias0[:], -1.0 / 16, 5.0,
                                            ALU.mult, ALU.add)
                    w0 = ap.tile([128, S], F8, tag="w0", name="w0")
                    nc.scalar.activation(w0[:, 0:512], psA, ACTF.Exp,
                                         bias=bias0[:, 0:1], scale=1.0 / 16)
                    nc.scalar.activation(w0[:, 512:1024], psB, ACTF.Exp,
                                         bias=bias0[:, 0:1], scale=1.0 / 16)
                    yield
                    # fp8 PE transposes write on 16-bit lanes: allocate the
                    # psum as bf16 and bitcast to an element-step-2 fp8 view
                    tps0 = ppt.tile([128, S], BF16, tag="tp", name="tps0")
                    tps08 = tps0[:].bitcast(F8).rearrange("p (a o) -> p a o", o=2)
                    for jc in range(NB):
                        nc.tensor.transpose(
                            tps08[:, jc * 128:(jc + 1) * 128, 0:1],
                            w0[:, jc * 128:(jc + 1) * 128], ident8[:])
                        if jc % 2 == 1:
                            yield
                    nc.scalar.activation(
                        wtT[:, :, 0:128],
                        tps08[:, :, 0:1].rearrange("p (a b) o -> p a b o", a=NB),
                        ACTF.Copy)
                    yield
                    if DEBUG and h == 0:
                        nc.sync.dma_start(out=dbg["w0"], in_=w0[:])

                def gen_ut(h):
                    """U^T[j,t] for t>=128 via fp8 DoubleRow; exp emits W^T."""
                    d = st[h]
                    a1, sqT, wtT = d["a1"], d["sqT"], d["wtT"]
                    sqA = sqT[:].rearrange("p (b c) -> p b c", c=1024)  # blocks 0-3
                    sqB = sqT[:].rearrange("p (b c) -> p b c", c=512)   # blocks 4-7
                    for jc in range(NB):
                        jsl = slice(jc * 128, (jc + 1) * 128)
                        p1 = ppm.tile([128, 384], F32, tag="mm", name="ps_ut1")
                        nc.tensor.matmul(p1[:], a1[:, 0:2, jsl], sqA[:, 0:2, 128:512],
                                         start=True, stop=False, perf_mode=DR)
                        nc.tensor.matmul(p1[:], a1[:, 2:4, jsl], sqA[:, 2:4, 128:512],
                                         start=False, stop=True, perf_mode=DR)
                        yield
                        nc.scalar.activation(wtT[:, jc, 128:512], p1[:], ACTF.Exp,
                                             scale=1.0 / 16)
                        p2 = ppm.tile([128, 512], F32, tag="mm", name="ps_ut2")
                        nc.tensor.matmul(p2[:], a1[:, 0:2, jsl], sqA[:, 0:2, 512:1024],
                                         start=True, stop=False, perf_mode=DR)
                        nc.tensor.matmul(p2[:], a1[:, 2:4, jsl], sqA[:, 2:4, 512:1024],
                                         start=False, stop=False, perf_mode=DR)
                        nc.tensor.matmul(p2[:], a1[:, 4:6, jsl], sqB[:, 8:10, :],
                                         start=False, stop=False, perf_mode=DR)
                        nc.tensor.matmul(p2[:], a1[:, 6:8, jsl], sqB[:, 10:12, :],
                                         start=False, stop=True, perf_mode=DR)
                        yield
                        nc.scalar.activation(wtT[:, jc, 512:1024], p2[:], ACTF.Exp,
                                             scale=1.0 / 16)
                    if DEBUG and h == 0:
                        nc.sync.dma_start(
                            out=dbg["wtT"].rearrange("p (a b) -> p a b", a=NB),
                            in_=wtT[:])

                def gen_a1t(h):
                    """A1^T via PE transposes of fp8 a1 + one copy per block."""
                    d = st[h]
                    a1 = d["a1"]
                    a1T = ap.tile([128, NB, S], F8, tag="a1t", bufs=1, name="a1T")
                    d["a1T"] = a1T
                    for m in range(NB):
                        tps = ppt.tile([128, S], BF16, tag="tp", name="tps")
                        tps8 = tps[:].bitcast(F8).rearrange("p (a o) -> p a o", o=2)
                        for k in range(NB):
                            nc.tensor.transpose(
                                tps8[:, k * 128:(k + 1) * 128, 0:1],
                                a1[:, m, k * 128:(k + 1) * 128], ident8[:])
                        yield
                        src = tps8[:, :, 0:1].rearrange("p (a b) o -> p a b o", a=NB)
                        if m == 7:
                            nc.scalar.activation(
                                a1T[:, :, m * 128:(m + 1) * 128], src, ACTF.Copy)
                        else:
                            nc.vector.tensor_copy(
                                a1T[:, :, m * 128:(m + 1) * 128], src)
                        yield

                def gen_den(h):
                    """den[t] = sum_j wtT[j,t] via DoubleRow x ones (free)."""
                    d = st[h]
                    wtT = d["wtT"]
                    dps = ppm.tile([128, NB], F32, tag="mm", name="ps_den")
                    for i in range(NB):
                        for k in range(4):
                            nc.tensor.matmul(
                                dps[:, i:i + 1],
                                wtT[:, 2 * k:2 * k + 2, i * 128:(i + 1) * 128],
                                ones8[:], start=(k == 0), stop=(k == 3),
                                perf_mode=DR)
                        if i % 2 == 1:
                            yield
                    denB = sp.tile([128, NB], F32, tag="denB", name="denB")
                    nc.vector.tensor_copy(denB[:], dps[:])
                    recden = sp.tile([128, NB], F32, tag="recden", name="recden")
                    nc.vector.reciprocal(recden[:], denB[:])
                    gsc = sp.tile([128, NB], F32, tag="gsc", name="gsc")
                    nc.vector.tensor_tensor(gsc[:], recden[:], invidx[:], ALU.mult)
                    d["gsc"] = gsc
                    yield
                    if DEBUG and h == 0:
                        nc.sync.dma_start(out=dbg["den"], in_=denB[:])

                def gen_s2(h):
                    """S2^T[s,t] = sum_j A1^T[j,s] W^T[j,t] (fp8 DoubleRow),
                    tril-masked on the diagonal, stored compact bf16."""
                    d = st[h]
                    a1T, wtT = d["a1T"], d["wtT"]
                    s2c = ap.tile([128, S2TOT], BF16, tag="s2", name="s2c")
                    d["s2c"] = s2c
                    for m in range(NB):
                        msl = slice(m * 128, (m + 1) * 128)
                        if m < 4:
                            chunks = [(m * 128, 512 - m * 128), (512, 512)]
                        else:
                            chunks = [(m * 128, 1024 - m * 128)]
                        for t0, w in chunks:
                            ps = ppm.tile([128, w], F32, tag="mm", name="ps_s2")
                            for k in range(4):
                                nc.tensor.matmul(
                                    ps[:], a1T[:, 2 * k:2 * k + 2, msl],
                                    wtT[:, 2 * k:2 * k + 2, t0:t0 + w],
                                    start=(k == 0), stop=(k == 3), perf_mode=DR)
                            yield
                            base = S2BASE[m] + (t0 - m * 128)
                            if t0 == m * 128:
                                nc.vector.tensor_tensor(
                                    s2c[:, base:base + 128], ps[:, 0:128],
                                    mask[:, 0, 0:128], ALU.mult)
                                if w > 128:
                                    nc.scalar.activation(
                                        s2c[:, base + 128:base + w], ps[:, 128:w],
                                        ACTF.Copy)
                            else:
                                nc.scalar.activation(s2c[:, base:base + w], ps[:],
                                                     ACTF.Copy)
                            yield
                    if DEBUG and h == 0:
                        nc.sync.dma_start(out=dbg["s2c"], in_=s2c[:])

                oNs = {}

                def emit_o(h):
                    """O[t,d] = gsc[t] * sum_{s<=t} S2T[s,t] vh[s,d]; heads
                    pair into one oN tile; PE transpose -> oT [d,t].  For the
                    last head the transpose + output projection are pipelined
                    per t-block to shrink the tail."""
                    d = st.pop(h)
                    s2c, gsc = d["s2c"], d["gsc"]
                    if h % 2 == 0:
                        oNs[h // 2] = sp.tile([128, NB, 128], BF16, tag="oN",
                                              bufs=4, name="oN")
                    oN = oNs[h // 2]
                    d0 = (h % 2) * 64
                    last = (h == HG - 1)
                    if not last:
                        # all 8 t-blocks share one psum bank (single
                        # accumulation group via the lazy zero-region), then
                        # one DVE multiply against a Pool-built broadcast gsc
                        gse = sp.tile([128, NB, 64], F32, tag="gse", name="gse")
                        nc.gpsimd.tensor_copy(
                            gse[:], gsc[:].broadcast_to((128, NB, 64)))
                        ps = ppm.tile([128, 512], F32, tag="mm", name="ps_o")
                        for i in range(NB):
                            for m in range(i + 1):
                                nc.tensor.matmul(
                                    ps[:, i * 64:(i + 1) * 64],
                                    s2c[:, S2BASE[m] + (i - m) * 128:
                                        S2BASE[m] + (i - m) * 128 + 128],
                                    vh[:, m, h * 64:(h + 1) * 64],
                                    start=(i == 0 and m == 0),
                                    stop=(i == NB - 1 and m == i),
                                    skip_group_check=True)
                            if i % 2 == 1:
                                yield
                        nc.vector.tensor_tensor(
                            oN[:, :, d0:d0 + 64],
                            ps[:].rearrange("p (a d) -> p a d", a=NB),
                            gse[:], ALU.mult)
                        yield
                    for i in range(NB if last else 0):
                        ps = ppm.tile([128, 64], F32, tag="mm", name="ps_o")
                        for m in range(i + 1):
                            nc.tensor.matmul(
                                ps[:], s2c[:, S2BASE[m] + (i - m) * 128:
                                           S2BASE[m] + (i - m) * 128 + 128],
                                vh[:, m, h * 64:(h + 1) * 64],
                                start=(m == 0), stop=(m == i))
                        nc.vector.tensor_scalar(oN[:, i, d0:d0 + 64], ps[:],
                                                gsc[:, i:i + 1], None, ALU.mult)
                        if last:
                            tps = ppm.tile([128, 128], BF16, tag="a1ps", bufs=2,
                                           name="tpo")
                            nc.tensor.transpose(tps[:], oN[:, i, :], ident[:])
                            nc.scalar.activation(
                                oT[:, h // 2, i * 128:(i + 1) * 128], tps[:],
                                ACTF.Copy)
                            # lag the output-projection tile one block so its
                            # matmuls never stall PE on the oT copy just above
                            if i >= 1:
                                emit_final_tile(i - 1)
                        yield
                    if last:
                        emit_final_tile(NB - 1)
                    if h % 2 == 1 and not last:
                        oN = oNs.pop(h // 2)
                        tps = ppm.tile([128, S], BF16, tag="mm", name="tpo")
                        for i in range(NB):
                            nc.tensor.transpose(
                                tps[:, i * 128:(i + 1) * 128], oN[:, i, :], ident[:])
                        nc.scalar.activation(
                            oT[:, h // 2, :],
                            tps[:].rearrange("p (a b) -> p a b", a=NB), ACTF.Copy)

                def emit_final_tile(i):
                    for c in range(2):
                        ps = ppm.tile([128, 512], F32, tag="mm", name="ps_fin")
                        for g2 in range(2):
                            nc.tensor.matmul(
                                ps[:], oT[:, g2, i * 128:(i + 1) * 128],
                                wct[:, g2, c * 512:(c + 1) * 512],
                                start=(g2 == 0), stop=(g2 == 1))
                        ot = sp.tile([128, 512], BF16, tag="ot", bufs=3, name="ot")
                        nc.vector.tensor_copy(ot[:], ps[:])
                        nc.sync.dma_start(
                            out=out_d[i * 128:(i + 1) * 128, c * 512:(c + 1) * 512],
                            in_=ot[:])

                def chain(*gens):
                    for gg in gens:
                        yield from gg

                # q/k projections interleaved with head 0's A1 matmuls
                # (which need only qhT+pTt) so the PE stream is never
                # head-of-line blocked on a projection DMA
                gen0 = gen_a1_sq(0)

                def pull0(k):
                    for _ in range(k):
                        if next(gen0, "done") == "done":
                            break

                for gi, (g, n) in enumerate(((0, 0), (0, 1), (1, 0), (1, 1))):
                    ps = ppm.tile([128, 512], F32, tag="mm", name="ps_proj")
                    for kb in range(NB):
                        nc.tensor.matmul(
                            ps[:], wqt[:, kb, g * 128:(g + 1) * 128],
                            qTt[:, kb, n * 512:(n + 1) * 512],
                            start=(kb == 0), stop=(kb == NB - 1))
                    nc.scalar.activation(
                        qhT[:, g, n * 512:(n + 1) * 512], ps[:],
                        ACTF.Identity, bias=wqb[:, g:g + 1], scale=NORM_D)
                    nc.gpsimd.tensor_tensor(
                        qsT[:, g, n * 512:(n + 1) * 512],
                        qhT[:, g, n * 512:(n + 1) * 512],
                        invrow[:, n, :], ALU.mult)
                    pull0((4, 4, 2, 2)[gi])
                for g in range(2):
                    for n in range(2):
                        ps = ppm.tile([128, 512], F32, tag="mm", name="ps_projk")
                        for kb in range(NB):
                            nc.tensor.matmul(
                                ps[:], wkt[:, kb, g * 128:(g + 1) * 128],
                                kTt[:, kb, n * 512:(n + 1) * 512],
                                start=(kb == 0), stop=(kb == NB - 1))
                        nc.scalar.activation(
                            khT[:, g, n * 512:(n + 1) * 512], ps[:],
                            ACTF.Identity, bias=wkb[:, g:g + 1], scale=1.0)
                        pull0(2)

                # vh[s, d] = sum_c vT[c, s] wv[c, d] + wv_b[d], interleaved
                # with head 0's A1/SqT so PE has work while vT streams in
                for m2 in range(0, NB, 2):
                    # two s-blocks share one psum bank: the first start marks
                    # the whole 2KB zero-region, the second block accumulates
                    # into its (lazily zeroed) half -> one copy per pair
                    ps = ppm.tile([128, 2 * DL], F32, tag="mm", name="ps_vh")
                    for mo in range(2):
                        m = m2 + mo
                        psl = ps[:, mo * DL:(mo + 1) * DL]
                        for kb in range(NB):
                            nc.tensor.matmul(
                                psl, vTt[:, kb, m * 128:(m + 1) * 128],
                                wvt[:, kb, :],
                                start=(m2 == m and kb == 0 and mo == 0),
                                stop=False, skip_group_check=True)
                        nc.tensor.matmul(psl, ones1[:], wvb[:], start=False,
                                         stop=(mo == 1), skip_group_check=True)
                        pull0(3)
                    nc.scalar.activation(vh[:, m2:m2 + 2, :],
                                         ps[:].rearrange("p (a d) -> p a d", a=2),
                                         ACTF.Copy)
                for _ in gen0:
                    pass

                for h in range(HG):
                    gnext = gen_a1_sq(h + 1) if h + 1 < HG else None
                    # O(h-1) first: its inputs are long-ready, so it fills
                    # the phase-boundary bubble while head h's a1/sqT fp8
                    # chains drain
                    parts = [emit_o(h - 1)] if h >= 1 else []
                    last = h == HG - 1
                    wgen = chain(*parts, gen_chunk0(h), gen_ut(h),
                                 *([] if last else [gen_a1t(h)]), gen_den(h))
                    for wi, _ in enumerate(wgen):
                        if gnext is not None:
                            next(gnext, None)
                            if wi < 12:
                                next(gnext, None)
                            if 6 <= wi < 10:
                                next(gnext, None)
                    if last:
                        # tail: S2's block m only needs a1T's m-chunk, and
                        # O's t-block i only needs s2c blocks m<=i -- so a1T
                        # production, S2, O and the output projection all
                        # pipeline per-block
                        a1tg = gen_a1t(h)
                        next(a1tg, None)
                        next(a1tg, None)
                        og = emit_o(h)
                        osteps = 0
                        yields_at_block_done = [4, 8, 12, 16, 18, 20, 22, 24]
                        nyield = 0
                        for _ in gen_s2(h):
                            nyield += 1
                            next(a1tg, None)
                            blocks_done = sum(
                                1 for yy in yields_at_block_done if nyield >= yy)
                            while og is not None and osteps < blocks_done:
                                if next(og, "done") == "done":
                                    og = None
                                    break
                                osteps += 1
                        if og is not None:
                            for _ in og:
                                pass
                    else:
                        for _ in gen_s2(h):
                            if gnext is not None:
                                next(gnext, None)
                        if gnext is not None:
                            for _ in gnext:
                                pass

            jp_cm.__exit__(None, None, None)
            vp_cm.__exit__(None, None, None)

    nc.finalize()
    return nc


_CACHE = {}


def _get_program():
    if "nc" not in _CACHE:
        _CACHE["nc"] = _build_program()
    return _CACHE["nc"]


def _consts():
    if "consts" not in _CACHE:
        p_ = np.arange(128, dtype=np.float32)[:, None]
        c_ = np.arange(512, dtype=np.float32)[None, :]
        mask4 = np.stack(
            [(p_ + 128.0 * r <= c_) for r in range(4)]).astype(NPBF)
        ident = np.eye(128, dtype=np.float32).astype(NPBF)
        ident8 = np.eye(128, dtype=np.float32).astype(NPF8)
        blk = np.arange(NB, dtype=np.float32)[None, :]
        invidx = (1.0 / (blk * 128.0 + p_ + 1.0)).astype(np.float32)
        ones1 = np.ones((1, 128), NPBF)
        ones8 = np.ones((128, 2), NPF8)
        n_ = np.arange(2, dtype=np.float32)[:, None]
        c2_ = np.arange(512, dtype=np.float32)[None, :]
        invrow = np.broadcast_to(
            (16.0 / (n_ * 512.0 + c2_ + 1.0))[None, :, :],
            (128, 2, 512)).astype(NPBF)
        invrow = np.ascontiguousarray(invrow)
        _CACHE["consts"] = (mask4, ident, ident8, invidx, ones1, ones8, invrow)
    return _CACHE["consts"]


PROFILE = False
LAST_RESULTS = None


def kernel(v, k, q, p, wq_k, wq_b, wk_k, wk_b, wv_k, wv_b, wc_k, wc_b):
    global LAST_RESULTS
    nc = _get_program()
    mask4, ident, ident8, invidx, ones1, ones8, invrow = _consts()

    qT = [np.ascontiguousarray(q[b].T).astype(NPBF) for b in range(B)]
    kT = [np.ascontiguousarray(k[b].T).astype(NPBF) for b in range(B)]
    vT = [np.ascontiguousarray(v[b].T).astype(NPBF) for b in range(B)]
    pT = [np.ascontiguousarray(p[b].T).astype(NPBF) for b in range(B)]
    wqc = wq_k.astype(NPBF)
    wkc = wk_k.astype(NPBF)
    wvc = wv_k.astype(NPBF)
    wcc = wc_k.astype(NPBF)

    in_maps = []
    for c in range(8):
        b, hg = c // 4, c % 4
        c0 = hg * DL
        wqb = np.ascontiguousarray(
            (wq_b[c0:c0 + DL].reshape(2, 128).T * NORM_D).astype(np.float32))
        wkb = np.ascontiguousarray(wk_b[c0:c0 + DL].reshape(2, 128).T.astype(np.float32))
        in_maps.append({
            "qT": qT[b], "kT": kT[b], "vT": vT[b],
            "pT": np.ascontiguousarray(pT[b][c0:c0 + DL]),
            "wq": np.ascontiguousarray(wqc[:, c0:c0 + DL]),
            "wk": np.ascontiguousarray(wkc[:, c0:c0 + DL]),
            "wv": np.ascontiguousarray(wvc[:, c0:c0 + DL]),
            "wc": np.ascontiguousarray(wcc[c0:c0 + DL, :]),
            "wqb": wqb, "wkb": wkb,
            "wvb": np.ascontiguousarray(wv_b[c0:c0 + DL].reshape(1, DL).astype(NPBF)),
            "ones1": ones1, "ones8": ones8, "mask4": mask4, "ident": ident,
            "ident8": ident8, "invidx": invidx, "invrow": invrow,
        })

    res = run_bass_kernel_spmd(
        nc, in_maps, core_ids=list(range(8)), trace=PROFILE)
    LAST_RESULTS = res

    out = np.zeros((B, S, DM), np.float32)
    for c in range(8):
        out[c // 4] += res.results[c]["out"].astype(np.float32)
    out += wc_b[None, None, :].astype(np.float32)
    return out
